# revision 10
# baseline (speedup 1.0000x reference)
"""Batched dynamic-filter cross-correlation on 8 Trainium2 NeuronCores.

Each sample b of x[128, 384, 384, 1] is VALID-correlated with its own
8x8 filter k[b] -> out[128, 377, 377, 1].

Strategy (pure data parallel, batch sharded 16 samples/core): the row
taps (p) contract on the TensorE partition dim via banded-Toeplitz
stationary matrices; the column taps (q) use a 2-parallel fast-FIR
(Karatsuba) decomposition to cut TensorE streaming work to 3/4:
  out[:, even] and out[:, odd] are recovered from three 4-tap
  half-rate sub-correlations P0 = H0*X0, P2 = H1*X1,
  P1 = (H0+H1)*(X0+X1), where X0/X1 are even/odd input columns
  (stride-2 access patterns, no copies) and H0/H1 even/odd taps of the
  q-reversed filter. Per 121-row output block this needs 12
  PSUM-accumulating matmuls of N=189 instead of 8 of N=378.
  out_even = P1 - P0 - P2, out_odd = shift(P0) + P2 are recombined by
  the vector/pool engines directly into the bf16 output tile.
The 14 leftover output rows of 4 samples are packed into one
block-diagonal 8-tap matmul group (K=4*21, M=4*14). All HBM tensors
are bf16 (accumulation stays fp32 in PSUM; host upcasts the output).
"""

import numpy as np
import ml_dtypes

BF16 = ml_dtypes.bfloat16

B, H, W = 128, 384, 384
KH, KW = 8, 8
HO, WO = H - KH + 1, W - KW + 1          # 377, 377
N_CORES = 8
SPC = B // N_CORES                        # 16 samples per core

MAIN_BLOCKS = [(0, 121, 128), (121, 121, 128), (242, 121, 128)]
TB, TM, TK = 363, 14, 21                  # tail rows: out 363..376, in 363..383
GS = 4                                    # tail-group size (samples per group)
NO2 = WO + 1                              # 378: tail moving width
XW = 386                                  # tail x tile width (q=7 reads col 384)
NP = 189                                  # half-rate sub-conv output cols (m=3..191)
XH = 192                                  # half-rate input length

_cache = {}


def _build_program():
    import concourse.mybir as mybir
    import concourse.tile as tile
    from concourse import bacc

    bf16 = mybir.dt.bfloat16
    f32 = mybir.dt.float32
    nc = bacc.Bacc(None, target_bir_lowering=False)
    x_d = nc.dram_tensor("x", [SPC, H, W], bf16, kind="ExternalInput")
    b_d = nc.dram_tensor("bands", [SPC, 128, 12, 121], bf16, kind="ExternalInput")
    t_d = nc.dram_tensor(
        "tailbands", [SPC // GS, GS * TK, KW, GS * TM], bf16, kind="ExternalInput"
    )
    o_d = nc.dram_tensor("out", [SPC, HO, WO], bf16, kind="ExternalOutput")

    with tile.TileContext(nc) as tc:
        with (
            tc.tile_pool(name="xp", bufs=6) as xp,
            tc.tile_pool(name="xsp", bufs=6) as xsp,
            tc.tile_pool(name="bp", bufs=3) as bp,
            tc.tile_pool(name="tbp", bufs=2) as tbp,
            tc.tile_pool(name="txp", bufs=2) as txp,
            tc.tile_pool(name="pa", bufs=2, space="PSUM") as pa,
            tc.tile_pool(name="pb", bufs=2, space="PSUM") as pb,
            tc.tile_pool(name="pc", bufs=2, space="PSUM") as pc,
            tc.tile_pool(name="pt", bufs=2, space="PSUM") as pt,
            tc.tile_pool(name="tp", bufs=6) as tp,
            tc.tile_pool(name="op", bufs=6) as op,
        ):
            for g in range(SPC // GS):
                for j in range(GS):
                    s = g * GS + j
                    bt = bp.tile([128, 12, 121], bf16)
                    nc.scalar.dma_start(out=bt[:], in_=b_d[s])
                    # prefetch all x blocks + X0+X1 pre-sums for this sample
                    xts, xss = [], []
                    for obase, M, K in MAIN_BLOCKS:
                        xt = xp.tile([128, W], bf16)
                        nc.gpsimd.dma_start(
                            out=xt[:K, :], in_=x_d[s, obase : obase + K, :]
                        )
                        xts.append(xt)
                    for bi, (obase, M, K) in enumerate(MAIN_BLOCKS):
                        xs = xsp.tile([128, XH], bf16)
                        xt = xts[bi]
                        nc.gpsimd.tensor_add(
                            out=xs[:K, :], in0=xt[:K, 0:W:2], in1=xt[:K, 1:W:2]
                        )
                        xss.append(xs)
                    for bi, (obase, M, K) in enumerate(MAIN_BLOCKS):
                        xt = xts[bi]
                        xs = xss[bi]
                        ps0 = pa.tile([128, 512], f32)
                        ps2 = pb.tile([128, 512], f32)
                        ps1 = pc.tile([128, 512], f32)
                        for u in range(4):
                            st = 2 * (3 - u)
                            nc.tensor.matmul(
                                ps0[:M, :NP],
                                bt[:K, u, :M],
                                xt[:K, st : st + 2 * NP - 1 : 2],
                                start=(u == 0),
                                stop=(u == 3),
                            )
                        for u in range(4):
                            st = 2 * (3 - u) + 1
                            nc.tensor.matmul(
                                ps2[:M, :NP],
                                bt[:K, 4 + u, :M],
                                xt[:K, st : st + 2 * NP - 1 : 2],
                                start=(u == 0),
                                stop=(u == 3),
                            )
                        for u in range(4):
                            st = 3 - u
                            nc.tensor.matmul(
                                ps1[:M, :NP],
                                bt[:K, 8 + u, :M],
                                xs[:K, st : st + NP],
                                start=(u == 0),
                                stop=(u == 3),
                            )
                        ot = op.tile([128, WO], bf16)
                        # out_even[t] = P1[t]-P0[t]-P2[t]; out_odd[t] = P0[t+1]+P2[t]
                        # (EW ops may read only one PSUM operand: stage P0 in SBUF)
                        c0 = tp.tile([128, NP], f32)
                        nc.scalar.copy(out=c0[:M, :], in_=ps0[:M, :NP])
                        c2 = tp.tile([128, NP], f32)
                        nc.scalar.copy(out=c2[:M, :], in_=ps2[:M, :NP])
                        t0 = tp.tile([128, NP], f32)
                        nc.vector.tensor_sub(
                            out=t0[:M, :], in0=ps1[:M, :NP], in1=c0[:M, :]
                        )
                        nc.vector.tensor_sub(
                            out=ot[:M, 0 : WO : 2], in0=t0[:M, :], in1=c2[:M, :]
                        )
                        nc.gpsimd.tensor_add(
                            out=ot[:M, 1 : WO : 2],
                            in0=c0[:M, 1:NP],
                            in1=c2[:M, 0 : NP - 1],
                        )
                        nc.sync.dma_start(
                            out=o_d[s, obase : obase + M, :], in_=ot[:M, :]
                        )
                # tail group: GS samples' last 14 rows, block-diagonal matmul
                tt = tbp.tile([GS * TK, KW, GS * TM], bf16)
                nc.scalar.dma_start(out=tt[:], in_=t_d[g])
                xtt = txp.tile([128, XW], bf16)
                nc.vector.memset(xtt[:, W:XW], 0.0)
                for j in range(GS):
                    nc.gpsimd.dma_start(
                        out=xtt[TK * j : TK * j + TK, :W],
                        in_=x_d[g * GS + j, TB : TB + TK, :],
                    )
                ps = pt.tile([128, 512], f32)
                for q in range(KW):
                    nc.tensor.matmul(
                        ps[: GS * TM, :NO2],
                        tt[: GS * TK, q, : GS * TM],
                        xtt[: GS * TK, q : q + NO2],
                        start=(q == 0),
                        stop=(q == KW - 1),
                    )
                ot = op.tile([128, WO], bf16)
                nc.scalar.copy(out=ot[: GS * TM, :], in_=ps[: GS * TM, :WO])
                for j in range(GS):
                    nc.sync.dma_start(
                        out=o_d[g * GS + j, TB : TB + TM, :],
                        in_=ot[TM * j : TM * j + TM, :],
                    )

    nc.compile()
    return nc


def _build_runner():
    """Build nc + a persistent jitted PJRT callable (compiles once)."""
    import jax
    from jax.sharding import Mesh, PartitionSpec
    from jax.experimental.shard_map import shard_map
    import concourse.mybir as mybir
    from concourse import bass2jax

    nc = _build_program()
    bass2jax.install_neuronx_cc_hook()

    partition_name = nc.partition_id_tensor.name if nc.partition_id_tensor else None

    in_names, out_names, out_avals, zero_shapes = [], [], [], []
    for alloc in nc.m.functions[0].allocations:
        if not isinstance(alloc, mybir.MemoryLocationSet):
            continue
        name = alloc.memorylocations[0].name
        if alloc.kind == "ExternalInput":
            if name != partition_name:
                in_names.append(name)
        elif alloc.kind == "ExternalOutput":
            shape = tuple(alloc.tensor_shape)
            dtype = mybir.dt.np(alloc.dtype)
            out_names.append(name)
            out_avals.append(jax.core.ShapedArray(shape, dtype))
            zero_shapes.append((shape, dtype))
    n_params = len(in_names)
    n_outs = len(out_avals)
    all_in_names = list(in_names) + list(out_names)
    if partition_name is not None:
        all_in_names.append(partition_name)

    def _body(*args):
        operands = list(args)
        if partition_name is not None:
            operands.append(bass2jax.partition_id_tensor())
        outs = bass2jax._bass_exec_p.bind(
            *operands,
            out_avals=tuple(out_avals),
            in_names=tuple(all_in_names),
            out_names=tuple(out_names),
            lowering_input_output_aliases=(),
            sim_require_finite=True,
            sim_require_nnan=True,
            nc=nc,
        )
        return tuple(outs)

    devices = jax.devices()[:N_CORES]
    mesh = Mesh(np.asarray(devices), ("core",))
    in_specs = (PartitionSpec("core"),) * (n_params + n_outs)
    out_specs = (PartitionSpec("core"),) * n_outs
    sharded = jax.jit(
        shard_map(
            _body, mesh=mesh, in_specs=in_specs, out_specs=out_specs, check_rep=False
        ),
        keep_unused=True,
    )

    from jax.sharding import NamedSharding

    zero_sharding = NamedSharding(mesh, PartitionSpec("core"))
    dev_zeros = [
        jax.device_put(np.zeros((N_CORES * s[0], *s[1:]), d), zero_sharding)
        for (s, d) in zero_shapes
    ]

    def run(in_maps):
        concat_in = [
            np.concatenate([np.asarray(m[name]) for m in in_maps], axis=0)
            for name in in_names
        ]
        out_arrs = sharded(*concat_in, *dev_zeros)
        return [
            {
                name: np.asarray(out_arrs[i]).reshape(
                    N_CORES, *out_avals[i].shape
                )[c]
                for i, name in enumerate(out_names)
            }
            for c in range(N_CORES)
        ]

    return nc, run


def _build_bands(k2):
    """k2: [B, 8, 8] fp32 -> Karatsuba bands [B, 128, 12, 121] bf16.

    h = q-reversed filter; H0/H1 = even/odd taps (4 each); planes
    0-3: Toeplitz bands of H0, 4-7: H1, 8-11: H0+H1.
    bands[b, m+p, plane(u), m] = Hx[b, p, u].
    """
    h = k2[:, :, ::-1]
    H0 = h[:, :, 0::2].astype(BF16)
    H1 = h[:, :, 1::2].astype(BF16)
    HS = (h[:, :, 0::2] + h[:, :, 1::2]).astype(BF16)
    bands = np.zeros((k2.shape[0], 128, 12, 121), BF16)
    m = np.arange(121)
    for p in range(KH):
        for u in range(4):
            bands[:, m + p, u, m] = H0[:, p, u][:, None]
            bands[:, m + p, 4 + u, m] = H1[:, p, u][:, None]
            bands[:, m + p, 8 + u, m] = HS[:, p, u][:, None]
    return bands


def _build_tailbands(k2):
    """k2: [N, 8, 8] -> block-diag tail bands [N//GS, GS*21, 8, GS*14]."""
    n = k2.shape[0]
    tb = np.zeros((n // GS, GS * TK, KW, GS * TM), BF16)
    m = np.arange(TM)
    k2 = k2.astype(BF16)
    for g in range(n // GS):
        for j in range(GS):
            for p in range(KH):
                tb[g, TK * j + m + p, :, TM * j + m] = k2[g * GS + j, p, :]
    return tb


def kernel(x, k):
    x = np.asarray(x, dtype=np.float32).reshape(B, H, W)
    k = np.asarray(k, dtype=np.float32).reshape(B, KH, KW)

    if "runner" not in _cache:
        _cache["runner"] = _build_runner()
    _nc, run = _cache["runner"]

    xb = x.astype(BF16)
    bands = _build_bands(k)
    tailbands = _build_tailbands(k)
    n_groups = SPC // GS
    in_maps = [
        {
            "x": np.ascontiguousarray(xb[c * SPC : (c + 1) * SPC]),
            "bands": bands[c * SPC : (c + 1) * SPC],
            "tailbands": tailbands[c * n_groups : (c + 1) * n_groups],
        }
        for c in range(N_CORES)
    ]
    results = run(in_maps)
    out = np.concatenate([r["out"] for r in results], axis=0)
    return out.astype(np.float32).reshape(B, HO, WO, 1)


# revision 12
# speedup vs baseline: 1.2296x; 1.2296x over previous
"""Batched dynamic-filter cross-correlation on 8 Trainium2 NeuronCores.

Each sample b of x[128, 384, 384, 1] is VALID-correlated with its own
8x8 filter k[b] -> out[128, 377, 377, 1].

Strategy (pure data parallel, batch sharded 16 samples/core): the row
taps (p) contract on the TensorE partition dim via banded-Toeplitz
stationary matrices; the column taps (q) use a 2-parallel fast-FIR
(Karatsuba) decomposition to cut TensorE streaming work to 3/4:
  out[:, even] and out[:, odd] are recovered from three 4-tap
  half-rate sub-correlations P0 = H0*X0, P2 = H1*X1,
  P1 = (H0+H1)*(X0+X1), where X0/X1 are even/odd input columns
  (stride-2 access patterns, no copies) and H0/H1 even/odd taps of the
  q-reversed filter. Per 121-row output block this needs 12
  PSUM-accumulating matmuls of N=189 instead of 8 of N=378.
  out_even = P1 - P0 - P2, out_odd = shift(P0) + P2 are recombined by
  the vector/pool engines directly into the bf16 output tile.
The 14 leftover output rows of 4 samples are packed into one
block-diagonal 8-tap matmul group (K=4*21, M=4*14). All HBM tensors
are bf16 (accumulation stays fp32 in PSUM; host upcasts the output).
"""

import numpy as np
import ml_dtypes

BF16 = ml_dtypes.bfloat16

B, H, W = 128, 384, 384
KH, KW = 8, 8
HO, WO = H - KH + 1, W - KW + 1          # 377, 377
N_CORES = 8
SPC = B // N_CORES                        # 16 samples per core

MAIN_BLOCKS = [(0, 121, 128), (121, 121, 128), (242, 121, 128)]
TB, TM, TK = 363, 14, 21                  # tail rows: out 363..376, in 363..383
GS = 4                                    # tail-group size (samples per group)
NO2 = WO + 1                              # 378: tail moving width
XW = 386                                  # tail x tile width (q=7 reads col 384)
NP = 189                                  # half-rate sub-conv output cols (m=3..191)
XH = 192                                  # half-rate input length

_cache = {}


def _build_program():
    import concourse.mybir as mybir
    import concourse.tile as tile
    from concourse import bacc

    bf16 = mybir.dt.bfloat16
    f32 = mybir.dt.float32
    nc = bacc.Bacc(None, target_bir_lowering=False)
    x_d = nc.dram_tensor("x", [SPC, H, W], bf16, kind="ExternalInput")
    b_d = nc.dram_tensor("bands", [SPC, 128, 12, 121], bf16, kind="ExternalInput")
    t_d = nc.dram_tensor(
        "tailbands", [SPC // GS, GS * TK, KW, GS * TM], bf16, kind="ExternalInput"
    )
    o_d = nc.dram_tensor("out", [SPC, HO, WO], bf16, kind="ExternalOutput")

    with tile.TileContext(nc) as tc:
        with (
            tc.tile_pool(name="xp", bufs=6) as xp,
            tc.tile_pool(name="xsp", bufs=6) as xsp,
            tc.tile_pool(name="bp", bufs=3) as bp,
            tc.tile_pool(name="tbp", bufs=2) as tbp,
            tc.tile_pool(name="txp", bufs=2) as txp,
            tc.tile_pool(name="pa", bufs=2, space="PSUM") as pa,
            tc.tile_pool(name="pb", bufs=2, space="PSUM") as pb,
            tc.tile_pool(name="pc", bufs=2, space="PSUM") as pc,
            tc.tile_pool(name="pt", bufs=2, space="PSUM") as pt,
            tc.tile_pool(name="tp", bufs=6) as tp,
            tc.tile_pool(name="op", bufs=6) as op,
        ):
            for g in range(SPC // GS):
                for j in range(GS):
                    s = g * GS + j
                    bt = bp.tile([128, 12, 121], bf16)
                    nc.scalar.dma_start(out=bt[:], in_=b_d[s])
                    # prefetch all x blocks + X0+X1 pre-sums for this sample
                    xts, xss = [], []
                    for obase, M, K in MAIN_BLOCKS:
                        xt = xp.tile([128, W], bf16)
                        nc.gpsimd.dma_start(
                            out=xt[:K, :], in_=x_d[s, obase : obase + K, :]
                        )
                        xts.append(xt)
                    for bi, (obase, M, K) in enumerate(MAIN_BLOCKS):
                        xs = xsp.tile([128, XH], bf16)
                        xt = xts[bi]
                        nc.vector.tensor_add(
                            out=xs[:K, :], in0=xt[:K, 0:W:2], in1=xt[:K, 1:W:2]
                        )
                        xss.append(xs)
                    for bi, (obase, M, K) in enumerate(MAIN_BLOCKS):
                        xt = xts[bi]
                        xs = xss[bi]
                        ps0 = pa.tile([128, 512], f32)
                        ps2 = pb.tile([128, 512], f32)
                        ps1 = pc.tile([128, 512], f32)
                        for u in range(4):
                            st = 2 * (3 - u)
                            nc.tensor.matmul(
                                ps0[:M, :NP],
                                bt[:K, u, :M],
                                xt[:K, st : st + 2 * NP - 1 : 2],
                                start=(u == 0),
                                stop=(u == 3),
                            )
                        for u in range(4):
                            st = 2 * (3 - u) + 1
                            nc.tensor.matmul(
                                ps2[:M, :NP],
                                bt[:K, 4 + u, :M],
                                xt[:K, st : st + 2 * NP - 1 : 2],
                                start=(u == 0),
                                stop=(u == 3),
                            )
                        for u in range(4):
                            st = 3 - u
                            nc.tensor.matmul(
                                ps1[:M, :NP],
                                bt[:K, 8 + u, :M],
                                xs[:K, st : st + NP],
                                start=(u == 0),
                                stop=(u == 3),
                            )
                        ot = op.tile([128, WO], bf16)
                        # out_even[t] = P1[t]-P0[t]-P2[t]; out_odd[t] = P0[t+1]+P2[t]
                        # (EW ops may read only one PSUM operand: stage P0 in SBUF)
                        c0 = tp.tile([128, NP], f32)
                        nc.scalar.copy(out=c0[:M, :], in_=ps0[:M, :NP])
                        t0 = tp.tile([128, NP], f32)
                        nc.vector.tensor_sub(
                            out=t0[:M, :], in0=ps1[:M, :NP], in1=c0[:M, :]
                        )
                        # odd cols = P0[t+1] + P2[t]
                        nc.vector.tensor_add(
                            out=ot[:M, 1 : WO : 2],
                            in0=c0[:M, 1:NP],
                            in1=ps2[:M, 0 : NP - 1],
                        )
                        nc.vector.tensor_sub(
                            out=ot[:M, 0 : WO : 2], in0=t0[:M, :], in1=ps2[:M, :NP]
                        )
                        nc.sync.dma_start(
                            out=o_d[s, obase : obase + M, :], in_=ot[:M, :]
                        )
                # tail group: GS samples' last 14 rows, block-diagonal matmul
                tt = tbp.tile([GS * TK, KW, GS * TM], bf16)
                nc.scalar.dma_start(out=tt[:], in_=t_d[g])
                xtt = txp.tile([128, XW], bf16)
                nc.vector.memset(xtt[:, W:XW], 0.0)
                for j in range(GS):
                    nc.gpsimd.dma_start(
                        out=xtt[TK * j : TK * j + TK, :W],
                        in_=x_d[g * GS + j, TB : TB + TK, :],
                    )
                ps = pt.tile([128, 512], f32)
                for q in range(KW):
                    nc.tensor.matmul(
                        ps[: GS * TM, :NO2],
                        tt[: GS * TK, q, : GS * TM],
                        xtt[: GS * TK, q : q + NO2],
                        start=(q == 0),
                        stop=(q == KW - 1),
                    )
                ot = op.tile([128, WO], bf16)
                nc.scalar.copy(out=ot[: GS * TM, :], in_=ps[: GS * TM, :WO])
                for j in range(GS):
                    nc.sync.dma_start(
                        out=o_d[g * GS + j, TB : TB + TM, :],
                        in_=ot[TM * j : TM * j + TM, :],
                    )

    nc.compile()
    return nc


def _build_runner():
    """Build nc + a persistent jitted PJRT callable (compiles once)."""
    import jax
    from jax.sharding import Mesh, PartitionSpec
    from jax.experimental.shard_map import shard_map
    import concourse.mybir as mybir
    from concourse import bass2jax

    nc = _build_program()
    bass2jax.install_neuronx_cc_hook()

    partition_name = nc.partition_id_tensor.name if nc.partition_id_tensor else None

    in_names, out_names, out_avals, zero_shapes = [], [], [], []
    for alloc in nc.m.functions[0].allocations:
        if not isinstance(alloc, mybir.MemoryLocationSet):
            continue
        name = alloc.memorylocations[0].name
        if alloc.kind == "ExternalInput":
            if name != partition_name:
                in_names.append(name)
        elif alloc.kind == "ExternalOutput":
            shape = tuple(alloc.tensor_shape)
            dtype = mybir.dt.np(alloc.dtype)
            out_names.append(name)
            out_avals.append(jax.core.ShapedArray(shape, dtype))
            zero_shapes.append((shape, dtype))
    n_params = len(in_names)
    n_outs = len(out_avals)
    all_in_names = list(in_names) + list(out_names)
    if partition_name is not None:
        all_in_names.append(partition_name)

    def _body(*args):
        operands = list(args)
        if partition_name is not None:
            operands.append(bass2jax.partition_id_tensor())
        outs = bass2jax._bass_exec_p.bind(
            *operands,
            out_avals=tuple(out_avals),
            in_names=tuple(all_in_names),
            out_names=tuple(out_names),
            lowering_input_output_aliases=(),
            sim_require_finite=True,
            sim_require_nnan=True,
            nc=nc,
        )
        return tuple(outs)

    devices = jax.devices()[:N_CORES]
    mesh = Mesh(np.asarray(devices), ("core",))
    in_specs = (PartitionSpec("core"),) * (n_params + n_outs)
    out_specs = (PartitionSpec("core"),) * n_outs
    sharded = jax.jit(
        shard_map(
            _body, mesh=mesh, in_specs=in_specs, out_specs=out_specs, check_rep=False
        ),
        keep_unused=True,
    )

    from jax.sharding import NamedSharding

    zero_sharding = NamedSharding(mesh, PartitionSpec("core"))
    dev_zeros = [
        jax.device_put(np.zeros((N_CORES * s[0], *s[1:]), d), zero_sharding)
        for (s, d) in zero_shapes
    ]

    def run(in_maps):
        concat_in = [
            np.concatenate([np.asarray(m[name]) for m in in_maps], axis=0)
            for name in in_names
        ]
        out_arrs = sharded(*concat_in, *dev_zeros)
        return [
            {
                name: np.asarray(out_arrs[i]).reshape(
                    N_CORES, *out_avals[i].shape
                )[c]
                for i, name in enumerate(out_names)
            }
            for c in range(N_CORES)
        ]

    return nc, run


def _build_bands(k2):
    """k2: [B, 8, 8] fp32 -> Karatsuba bands [B, 128, 12, 121] bf16.

    h = q-reversed filter; H0/H1 = even/odd taps (4 each); planes
    0-3: Toeplitz bands of H0, 4-7: H1, 8-11: H0+H1.
    bands[b, m+p, plane(u), m] = Hx[b, p, u].
    """
    h = k2[:, :, ::-1]
    H0 = h[:, :, 0::2].astype(BF16)
    H1 = h[:, :, 1::2].astype(BF16)
    HS = (h[:, :, 0::2] + h[:, :, 1::2]).astype(BF16)
    bands = np.zeros((k2.shape[0], 128, 12, 121), BF16)
    m = np.arange(121)
    for p in range(KH):
        for u in range(4):
            bands[:, m + p, u, m] = H0[:, p, u][:, None]
            bands[:, m + p, 4 + u, m] = H1[:, p, u][:, None]
            bands[:, m + p, 8 + u, m] = HS[:, p, u][:, None]
    return bands


def _build_tailbands(k2):
    """k2: [N, 8, 8] -> block-diag tail bands [N//GS, GS*21, 8, GS*14]."""
    n = k2.shape[0]
    tb = np.zeros((n // GS, GS * TK, KW, GS * TM), BF16)
    m = np.arange(TM)
    k2 = k2.astype(BF16)
    for g in range(n // GS):
        for j in range(GS):
            for p in range(KH):
                tb[g, TK * j + m + p, :, TM * j + m] = k2[g * GS + j, p, :]
    return tb


def kernel(x, k):
    x = np.asarray(x, dtype=np.float32).reshape(B, H, W)
    k = np.asarray(k, dtype=np.float32).reshape(B, KH, KW)

    if "runner" not in _cache:
        _cache["runner"] = _build_runner()
    _nc, run = _cache["runner"]

    xb = x.astype(BF16)
    bands = _build_bands(k)
    tailbands = _build_tailbands(k)
    n_groups = SPC // GS
    in_maps = [
        {
            "x": np.ascontiguousarray(xb[c * SPC : (c + 1) * SPC]),
            "bands": bands[c * SPC : (c + 1) * SPC],
            "tailbands": tailbands[c * n_groups : (c + 1) * n_groups],
        }
        for c in range(N_CORES)
    ]
    results = run(in_maps)
    out = np.concatenate([r["out"] for r in results], axis=0)
    return out.astype(np.float32).reshape(B, HO, WO, 1)


# revision 24
# speedup vs baseline: 1.8805x; 1.5294x over previous
"""Batched dynamic-filter cross-correlation on 8 Trainium2 NeuronCores.

Each sample b of x[128, 384, 384, 1] is VALID-correlated with its own
8x8 filter k[b] -> out[128, 377, 377, 1].

Strategy (pure data parallel, batch sharded 16 samples/core): the row
taps (p) contract on the TensorE partition dim via banded-Toeplitz
stationary matrices; the column taps (q) use a 2-parallel fast-FIR
(Karatsuba) decomposition to cut TensorE streaming work to 3/4:
  out[:, even] and out[:, odd] are recovered from three 4-tap
  half-rate sub-correlations P0 = H0*X0, P2 = H1*X1,
  P1 = (H0+H1)*(X0+X1), where X0/X1 are even/odd input columns
  (stride-2 access patterns, no copies) and H0/H1 even/odd taps of the
  q-reversed filter. Per 121-row output block this needs 12
  PSUM-accumulating matmuls of N=189 instead of 8 of N=378.
  out_even = P1 - P0 - P2, out_odd = shift(P0) + P2 are recombined by
  the vector/pool engines directly into the bf16 output tile.
The 14 leftover output rows of 4 samples are packed into one
block-diagonal 8-tap matmul group (K=4*21, M=4*14). All HBM tensors
are bf16 (accumulation stays fp32 in PSUM; host upcasts the output).
"""

import numpy as np
import ml_dtypes

BF16 = ml_dtypes.bfloat16

B, H, W = 128, 384, 384
KH, KW = 8, 8
HO, WO = H - KH + 1, W - KW + 1          # 377, 377
N_CORES = 8
SPC = B // N_CORES                        # 16 samples per core

MAIN_BLOCKS = [(0, 121, 128), (121, 121, 128), (242, 121, 128)]
TB, TM, TK = 363, 14, 21                  # tail rows: out 363..376, in 363..383
GS = 4                                    # tail-group size (samples per group)
NO2 = WO + 1                              # 378: tail moving width
XW = 386                                  # tail x tile width (q=7 reads col 384)
NP = 189                                  # half-rate sub-conv output cols (m=3..191)
XH = 192                                  # half-rate input length

_cache = {}


def _build_program():
    import concourse.mybir as mybir
    import concourse.tile as tile
    from concourse import bacc

    from concourse.ap import AP

    bf16 = mybir.dt.bfloat16
    f32 = mybir.dt.float32

    def win3(base, nrows, width, step):
        """[128, 3, width] view of a [H|HO, width] DRAM sample: row = step*t + kk."""
        return AP(base.tensor, base.offset,
                  [[width, nrows], [step * width, 3], [1, width]])

    nc = bacc.Bacc(None, target_bir_lowering=False)
    x_d = nc.dram_tensor("x", [SPC, H, W + XH], bf16, kind="ExternalInput")
    b_d = nc.dram_tensor("bands", [SPC, 128, 12, 121], bf16, kind="ExternalInput")
    t_d = nc.dram_tensor(
        "tailbands", [SPC // GS, GS * TK, KW, GS * TM], bf16, kind="ExternalInput"
    )
    o_d = nc.dram_tensor("out", [SPC, HO, WO], bf16, kind="ExternalOutput")

    with tile.TileContext(nc) as tc:
        with (
            tc.tile_pool(name="xp", bufs=3) as xp,
            tc.tile_pool(name="bp", bufs=2) as bp,
            tc.tile_pool(name="tbp", bufs=1) as tbp,
            tc.tile_pool(name="txp", bufs=2) as txp,
            tc.tile_pool(name="pa", bufs=2, space="PSUM") as pa,
            tc.tile_pool(name="pb", bufs=2, space="PSUM") as pb,
            tc.tile_pool(name="pc", bufs=2, space="PSUM") as pc,
            tc.tile_pool(name="pt", bufs=2, space="PSUM") as pt,
            tc.tile_pool(name="tp", bufs=6) as tp,
            tc.tile_pool(name="op", bufs=3) as op,
        ):
            # x prefetch rolls one sample ahead; bands are per-sample
            xts = {}
            xt0 = xp.tile([128, 3, W + XH], bf16, name="xtn")
            xts[0] = xt0
            tt = None
            bts = {}
            for g in range(SPC // GS):
                if g == 0:
                    bta = bp.tile([128, 1, 12, 121], bf16, name="bta")
                    nc.scalar.dma_start(out=bta[:, 0, 0:4], in_=b_d[0, :, 0:4])
                    nc.sync.dma_start(out=xt0[:, 0], in_=x_d[0, 0:128])
                    nc.scalar.dma_start(out=bta[:, 0, 4:12], in_=b_d[0, :, 4:12])
                    nc.sync.dma_start(out=xt0[:, 1], in_=x_d[0, 121:249])
                    nc.sync.dma_start(out=xt0[:, 2], in_=x_d[0, 242:370])
                    bt = bp.tile([128, GS, 12, 121], bf16, name="bt")
                    bts[0] = bt
                    nc.sync.dma_start(
                        out=bt[:, 1:],
                        in_=b_d[1:GS].transpose([1, 0, 2, 3]),
                    )
                bt = bts.pop(g)
                xtt = txp.tile([GS * TK, W], bf16)
                nc.sync.dma_start(
                    out=xtt[:], in_=x_d[g * GS : (g + 1) * GS, TB : TB + TK, :W]
                )
                for j in range(GS):
                    s = g * GS + j
                    if s + 1 < SPC:
                        xtn = xp.tile([128, 3, W + XH], bf16, name="xtn")
                        xts[s + 1] = xtn
                        nc.sync.dma_start(
                            out=xtn[:], in_=win3(x_d[s + 1], 128, W + XH, 121)
                        )
                    if tt is None:
                        tt = tbp.tile([GS * TK, SPC // GS, KW, GS * TM], bf16)
                        nc.sync.dma_start(out=tt[:], in_=t_d[:].transpose([1, 0, 2, 3]))
                    xt = xts.pop(s)
                    btj = bta[:, 0] if s == 0 else bt[:, j]
                    ot = op.tile([121, 3, WO], bf16)
                    for bi, (obase, M, K) in enumerate(MAIN_BLOCKS):
                        ps0 = pa.tile([128, 512], f32)
                        ps2 = pb.tile([128, 512], f32)
                        ps1 = pc.tile([128, 512], f32)
                        for u in range(4):
                            st = 2 * (3 - u)
                            nc.tensor.matmul(
                                ps0[:M, :NP],
                                btj[:K, u, :M],
                                xt[:K, bi, st : st + 2 * NP - 1 : 2],
                                start=(u == 0),
                                stop=(u == 3),
                            )
                        for u in range(4):
                            st = 2 * (3 - u) + 1
                            nc.tensor.matmul(
                                ps2[:M, :NP],
                                btj[:K, 4 + u, :M],
                                xt[:K, bi, st : st + 2 * NP - 1 : 2],
                                start=(u == 0),
                                stop=(u == 3),
                            )
                        for u in range(4):
                            st = W + 3 - u
                            nc.tensor.matmul(
                                ps1[:M, :NP],
                                btj[:K, 8 + u, :M],
                                xt[:K, bi, st : st + NP],
                                start=(u == 0),
                                stop=(u == 3),
                            )
                        # stage P0/P2 in SBUF (Act), recombine on DVE:
                        # even = P1-P0-P2, odd[t] = P0[t+1]+P2[t]
                        c0 = tp.tile([128, NP], f32)
                        nc.scalar.copy(out=c0[:M, :], in_=ps0[:M, :NP])
                        c2 = tp.tile([128, NP], f32)
                        nc.scalar.copy(out=c2[:M, :], in_=ps2[:M, :NP])
                        nc.vector.tensor_add(
                            out=ot[:M, bi, 1 : WO : 2],
                            in0=c0[:M, 1:NP],
                            in1=c2[:M, 0 : NP - 1],
                        )
                        t0 = tp.tile([128, NP], f32)
                        nc.vector.tensor_sub(
                            out=t0[:M, :], in0=ps1[:M, :NP], in1=c0[:M, :]
                        )
                        nc.vector.tensor_sub(
                            out=ot[:M, bi, 0 : WO : 2], in0=t0[:M, :], in1=c2[:M, :]
                        )
                    if s == SPC - 1:
                        nc.sync.dma_start(
                            out=win3(o_d[s], 121, WO, 121), in_=ot[:]
                        )
                    else:
                        nc.gpsimd.dma_start(
                            out=win3(o_d[s], 121, WO, 121), in_=ot[:]
                        )
                    if j == 2 and g + 1 < SPC // GS:
                        g2 = g + 1
                        btn = bp.tile([128, GS, 12, 121], bf16, name="bt")
                        bts[g2] = btn
                        nc.sync.dma_start(
                            out=btn[:],
                            in_=b_d[g2 * GS : (g2 + 1) * GS].transpose([1, 0, 2, 3]),
                        )
                    if j == 1:
                        # tail: GS samples' last 14 rows, block-diagonal matmul
                        ps = pt.tile([128, 512], f32)
                        for q in range(KW):
                            nc.tensor.matmul(
                                ps[: GS * TM, :WO],
                                tt[: GS * TK, g, q, : GS * TM],
                                xtt[: GS * TK, q : q + WO],
                                start=(q == 0),
                                stop=(q == KW - 1),
                            )
                        oto = op.tile([121, 3, WO], bf16)
                        nc.scalar.copy(
                            out=oto[: GS * TM, 0, :], in_=ps[: GS * TM, :WO]
                        )
                        nc.gpsimd.dma_start(
                            out=o_d[g * GS : (g + 1) * GS, TB : TB + TM, :],
                            in_=oto[: GS * TM, 0, :],
                        )

    nc.compile()
    return nc


def _build_runner():
    """Build nc + a persistent jitted PJRT callable (compiles once)."""
    import jax
    from jax.sharding import Mesh, PartitionSpec
    from jax.experimental.shard_map import shard_map
    import concourse.mybir as mybir
    from concourse import bass2jax

    nc = _build_program()
    bass2jax.install_neuronx_cc_hook()

    partition_name = nc.partition_id_tensor.name if nc.partition_id_tensor else None

    in_names, out_names, out_avals, zero_shapes = [], [], [], []
    for alloc in nc.m.functions[0].allocations:
        if not isinstance(alloc, mybir.MemoryLocationSet):
            continue
        name = alloc.memorylocations[0].name
        if alloc.kind == "ExternalInput":
            if name != partition_name:
                in_names.append(name)
        elif alloc.kind == "ExternalOutput":
            shape = tuple(alloc.tensor_shape)
            dtype = mybir.dt.np(alloc.dtype)
            out_names.append(name)
            out_avals.append(jax.core.ShapedArray(shape, dtype))
            zero_shapes.append((shape, dtype))
    n_params = len(in_names)
    n_outs = len(out_avals)
    all_in_names = list(in_names) + list(out_names)
    if partition_name is not None:
        all_in_names.append(partition_name)

    def _body(*args):
        operands = list(args)
        if partition_name is not None:
            operands.append(bass2jax.partition_id_tensor())
        outs = bass2jax._bass_exec_p.bind(
            *operands,
            out_avals=tuple(out_avals),
            in_names=tuple(all_in_names),
            out_names=tuple(out_names),
            lowering_input_output_aliases=(),
            sim_require_finite=True,
            sim_require_nnan=True,
            nc=nc,
        )
        return tuple(outs)

    devices = jax.devices()[:N_CORES]
    mesh = Mesh(np.asarray(devices), ("core",))
    in_specs = (PartitionSpec("core"),) * (n_params + n_outs)
    out_specs = (PartitionSpec("core"),) * n_outs
    sharded = jax.jit(
        shard_map(
            _body, mesh=mesh, in_specs=in_specs, out_specs=out_specs, check_rep=False
        ),
        keep_unused=True,
    )

    from jax.sharding import NamedSharding

    zero_sharding = NamedSharding(mesh, PartitionSpec("core"))
    dev_zeros = [
        jax.device_put(np.zeros((N_CORES * s[0], *s[1:]), d), zero_sharding)
        for (s, d) in zero_shapes
    ]

    def run(in_maps):
        concat_in = [
            np.concatenate([np.asarray(m[name]) for m in in_maps], axis=0)
            for name in in_names
        ]
        out_arrs = sharded(*concat_in, *dev_zeros)
        return [
            {
                name: np.asarray(out_arrs[i]).reshape(
                    N_CORES, *out_avals[i].shape
                )[c]
                for i, name in enumerate(out_names)
            }
            for c in range(N_CORES)
        ]

    return nc, run


def _build_bands(k2):
    """k2: [B, 8, 8] fp32 -> Karatsuba bands [B, 128, 12, 121] bf16.

    h = q-reversed filter; H0/H1 = even/odd taps (4 each); planes
    0-3: Toeplitz bands of H0, 4-7: H1, 8-11: H0+H1.
    bands[b, m+p, plane(u), m] = Hx[b, p, u].
    """
    h = k2[:, :, ::-1]
    H0 = h[:, :, 0::2].astype(BF16)
    H1 = h[:, :, 1::2].astype(BF16)
    HS = (h[:, :, 0::2] + h[:, :, 1::2]).astype(BF16)
    bands = np.zeros((k2.shape[0], 128, 12, 121), BF16)
    m = np.arange(121)
    for p in range(KH):
        for u in range(4):
            bands[:, m + p, u, m] = H0[:, p, u][:, None]
            bands[:, m + p, 4 + u, m] = H1[:, p, u][:, None]
            bands[:, m + p, 8 + u, m] = HS[:, p, u][:, None]
    return bands


def _build_tailbands(k2):
    """k2: [N, 8, 8] -> block-diag tail bands [N//GS, GS*21, 8, GS*14]."""
    n = k2.shape[0]
    tb = np.zeros((n // GS, GS * TK, KW, GS * TM), BF16)
    m = np.arange(TM)
    k2 = k2.astype(BF16)
    for g in range(n // GS):
        for j in range(GS):
            for p in range(KH):
                tb[g, TK * j + m + p, :, TM * j + m] = k2[g * GS + j, p, :]
    return tb


def kernel(x, k):
    x = np.asarray(x, dtype=np.float32).reshape(B, H, W)
    k = np.asarray(k, dtype=np.float32).reshape(B, KH, KW)

    if "runner" not in _cache:
        _cache["runner"] = _build_runner()
    _nc, run = _cache["runner"]

    xs = x[:, :, 0::2] + x[:, :, 1::2]
    xb = np.concatenate([x, xs], axis=2).astype(BF16)
    bands = _build_bands(k)
    tailbands = _build_tailbands(k)
    n_groups = SPC // GS
    in_maps = [
        {
            "x": np.ascontiguousarray(xb[c * SPC : (c + 1) * SPC]),
            "bands": bands[c * SPC : (c + 1) * SPC],
            "tailbands": tailbands[c * n_groups : (c + 1) * n_groups],
        }
        for c in range(N_CORES)
    ]
    results = run(in_maps)
    out = np.concatenate([r["out"] for r in results], axis=0)
    return out.astype(np.float32).reshape(B, HO, WO, 1)


# revision 26
# speedup vs baseline: 1.8987x; 1.0097x over previous
"""Batched dynamic-filter cross-correlation on 8 Trainium2 NeuronCores.

Each sample b of x[128, 384, 384, 1] is VALID-correlated with its own
8x8 filter k[b] -> out[128, 377, 377, 1].

Strategy (pure data parallel, batch sharded 16 samples/core): the row
taps (p) contract on the TensorE partition dim via banded-Toeplitz
stationary matrices; the column taps (q) use a 2-parallel fast-FIR
(Karatsuba) decomposition that cuts TensorE streaming work to 3/4 of
direct: three 4-tap half-rate sub-correlations P0 = H0*X0, P2 = H1*X1,
P1 = (H0+H1)*(X0+X1), where X0/X1 are even/odd input columns (read
with stride-2 access patterns straight from the x tile) and H0/H1 are
even/odd taps of the q-reversed filter. Per 121-row output block this
is 12 PSUM-accumulating matmuls of N=189 instead of 8 of N=378.
Recombination (even = P1-P0-P2, odd[t] = P0[t+1]+P2[t]) runs on the
Activation engine (PSUM->SBUF stages of P0/P2) and the vector engine
(strided bf16 writes into the output tile).

Everything in HBM is bf16 (fp32 accumulation in PSUM; the host
upcasts): x ships as rows [x | X0+X1] (the pre-sum is computed on the
host and packed contiguously so DMA lines stay >= 512B), bands hold
the 12 Toeplitz tap-planes. DMA instruction count is minimized (one
x DMA per sample via an overlapping 3-window access pattern, one
banded-weights DMA per 4-sample group prefetched two samples early,
one output DMA per sample) because each DMA instruction serializes
~0.6-1us on the shared HWDGE/SWDGE descriptor generators; output DMAs
ride the otherwise-idle Pool SWDGE queue. The 14 leftover output rows
of 4 samples are packed into one block-diagonal 8-tap matmul group
(K=4*21, M=4*14) scheduled mid-group to stay off the critical path.
"""

import numpy as np
import ml_dtypes

BF16 = ml_dtypes.bfloat16

B, H, W = 128, 384, 384
KH, KW = 8, 8
HO, WO = H - KH + 1, W - KW + 1          # 377, 377
N_CORES = 8
SPC = B // N_CORES                        # 16 samples per core

MAIN_BLOCKS = [(0, 121, 128), (121, 121, 128), (242, 121, 128)]
TB, TM, TK = 363, 14, 21                  # tail rows: out 363..376, in 363..383
GS = 4                                    # tail-group size (samples per group)
NO2 = WO + 1                              # 378: tail moving width
XW = 386                                  # tail x tile width (q=7 reads col 384)
NP = 189                                  # half-rate sub-conv output cols (m=3..191)
XH = 192                                  # half-rate input length

_cache = {}


def _build_program():
    import concourse.mybir as mybir
    import concourse.tile as tile
    from concourse import bacc

    from concourse.ap import AP

    bf16 = mybir.dt.bfloat16
    f32 = mybir.dt.float32

    def win3(base, nrows, width, step):
        """[128, 3, width] view of a [H|HO, width] DRAM sample: row = step*t + kk."""
        return AP(base.tensor, base.offset,
                  [[width, nrows], [step * width, 3], [1, width]])

    nc = bacc.Bacc(None, target_bir_lowering=False)
    x_d = nc.dram_tensor("x", [SPC, H, W + XH], bf16, kind="ExternalInput")
    b_d = nc.dram_tensor("bands", [SPC, 128, 12, 121], bf16, kind="ExternalInput")
    t_d = nc.dram_tensor(
        "tailbands", [SPC // GS, GS * TK, KW, GS * TM], bf16, kind="ExternalInput"
    )
    o_d = nc.dram_tensor("out", [SPC, HO, WO], bf16, kind="ExternalOutput")

    with tile.TileContext(nc) as tc:
        with (
            tc.tile_pool(name="xp", bufs=3) as xp,
            tc.tile_pool(name="bp", bufs=2) as bp,
            tc.tile_pool(name="tbp", bufs=1) as tbp,
            tc.tile_pool(name="txp", bufs=2) as txp,
            tc.tile_pool(name="pa", bufs=2, space="PSUM") as pa,
            tc.tile_pool(name="pb", bufs=2, space="PSUM") as pb,
            tc.tile_pool(name="pc", bufs=2, space="PSUM") as pc,
            tc.tile_pool(name="pt", bufs=2, space="PSUM") as pt,
            tc.tile_pool(name="tp", bufs=6) as tp,
            tc.tile_pool(name="op", bufs=3) as op,
        ):
            # x prefetch rolls one sample ahead; bands are per-sample
            xts = {}
            xt0 = xp.tile([128, 3, W + XH], bf16, name="xtn")
            xts[0] = xt0
            tt = None
            bts = {}
            for g in range(SPC // GS):
                if g == 0:
                    bta = bp.tile([128, 1, 12, 121], bf16, name="bta")
                    nc.scalar.dma_start(out=bta[:, 0, 0:4], in_=b_d[0, :, 0:4])
                    nc.sync.dma_start(out=xt0[:, 0], in_=x_d[0, 0:128])
                    nc.scalar.dma_start(out=bta[:, 0, 4:12], in_=b_d[0, :, 4:12])
                    nc.sync.dma_start(out=xt0[:, 1], in_=x_d[0, 121:249])
                    nc.sync.dma_start(out=xt0[:, 2], in_=x_d[0, 242:370])
                    bt = bp.tile([128, GS, 12, 121], bf16, name="bt")
                    bts[0] = bt
                    nc.sync.dma_start(
                        out=bt[:, 1:],
                        in_=b_d[1:GS].transpose([1, 0, 2, 3]),
                    )
                bt = bts.pop(g)
                xtt = txp.tile([GS * TK, W], bf16)
                nc.sync.dma_start(
                    out=xtt[:], in_=x_d[g * GS : (g + 1) * GS, TB : TB + TK, :W]
                )
                for j in range(GS):
                    s = g * GS + j
                    if s + 1 < SPC:
                        xtn = xp.tile([128, 3, W + XH], bf16, name="xtn")
                        xts[s + 1] = xtn
                        nc.sync.dma_start(
                            out=xtn[:], in_=win3(x_d[s + 1], 128, W + XH, 121)
                        )
                    if tt is None:
                        tt = tbp.tile([GS * TK, SPC // GS, KW, GS * TM], bf16)
                        nc.sync.dma_start(out=tt[:], in_=t_d[:].transpose([1, 0, 2, 3]))
                    xt = xts.pop(s)
                    btj = bta[:, 0] if s == 0 else bt[:, j]
                    ot = op.tile([121, 3, WO], bf16)
                    for bi, (obase, M, K) in enumerate(MAIN_BLOCKS):
                        ps0 = pa.tile([128, 512], f32)
                        ps2 = pb.tile([128, 512], f32)
                        ps1 = pc.tile([128, 512], f32)
                        for u in range(4):
                            st = 2 * (3 - u)
                            nc.tensor.matmul(
                                ps0[:M, :NP],
                                btj[:K, u, :M],
                                xt[:K, bi, st : st + 2 * NP - 1 : 2],
                                start=(u == 0),
                                stop=(u == 3),
                            )
                        for u in range(4):
                            st = 2 * (3 - u) + 1
                            nc.tensor.matmul(
                                ps2[:M, :NP],
                                btj[:K, 4 + u, :M],
                                xt[:K, bi, st : st + 2 * NP - 1 : 2],
                                start=(u == 0),
                                stop=(u == 3),
                            )
                        for u in range(4):
                            st = W + 3 - u
                            nc.tensor.matmul(
                                ps1[:M, :NP],
                                btj[:K, 8 + u, :M],
                                xt[:K, bi, st : st + NP],
                                start=(u == 0),
                                stop=(u == 3),
                            )
                        # stage P0/P2 in SBUF (Act), recombine on DVE:
                        # even = P1-P0-P2, odd[t] = P0[t+1]+P2[t]
                        c0 = tp.tile([128, NP], f32)
                        nc.scalar.copy(out=c0[:M, :], in_=ps0[:M, :NP])
                        c2 = tp.tile([128, NP], f32)
                        nc.scalar.copy(out=c2[:M, :], in_=ps2[:M, :NP])
                        nc.vector.tensor_add(
                            out=ot[:M, bi, 1 : WO : 2],
                            in0=c0[:M, 1:NP],
                            in1=c2[:M, 0 : NP - 1],
                        )
                        t0 = tp.tile([128, NP], f32)
                        nc.vector.tensor_sub(
                            out=t0[:M, :], in0=ps1[:M, :NP], in1=c0[:M, :]
                        )
                        nc.vector.tensor_sub(
                            out=ot[:M, bi, 0 : WO : 2], in0=t0[:M, :], in1=c2[:M, :]
                        )
                    if s == SPC - 1:
                        for bi2, (ob2, M2, _K2) in enumerate(MAIN_BLOCKS):
                            nc.sync.dma_start(
                                out=o_d[s, ob2 : ob2 + M2, :],
                                in_=ot[:M2, bi2, :],
                            )
                    else:
                        nc.gpsimd.dma_start(
                            out=win3(o_d[s], 121, WO, 121), in_=ot[:]
                        )
                    if j == 2 and g + 1 < SPC // GS:
                        g2 = g + 1
                        btn = bp.tile([128, GS, 12, 121], bf16, name="bt")
                        bts[g2] = btn
                        nc.sync.dma_start(
                            out=btn[:],
                            in_=b_d[g2 * GS : (g2 + 1) * GS].transpose([1, 0, 2, 3]),
                        )
                    if j == 1:
                        # tail: GS samples' last 14 rows, block-diagonal matmul
                        ps = pt.tile([128, 512], f32)
                        for q in range(KW):
                            nc.tensor.matmul(
                                ps[: GS * TM, :WO],
                                tt[: GS * TK, g, q, : GS * TM],
                                xtt[: GS * TK, q : q + WO],
                                start=(q == 0),
                                stop=(q == KW - 1),
                            )
                        oto = op.tile([121, 3, WO], bf16)
                        nc.scalar.copy(
                            out=oto[: GS * TM, 0, :], in_=ps[: GS * TM, :WO]
                        )
                        nc.gpsimd.dma_start(
                            out=o_d[g * GS : (g + 1) * GS, TB : TB + TM, :],
                            in_=oto[: GS * TM, 0, :],
                        )

    nc.compile()
    return nc


def _build_runner():
    """Build nc + a persistent jitted PJRT callable (compiles once)."""
    import jax
    from jax.sharding import Mesh, PartitionSpec
    from jax.experimental.shard_map import shard_map
    import concourse.mybir as mybir
    from concourse import bass2jax

    nc = _build_program()
    bass2jax.install_neuronx_cc_hook()

    partition_name = nc.partition_id_tensor.name if nc.partition_id_tensor else None

    in_names, out_names, out_avals, zero_shapes = [], [], [], []
    for alloc in nc.m.functions[0].allocations:
        if not isinstance(alloc, mybir.MemoryLocationSet):
            continue
        name = alloc.memorylocations[0].name
        if alloc.kind == "ExternalInput":
            if name != partition_name:
                in_names.append(name)
        elif alloc.kind == "ExternalOutput":
            shape = tuple(alloc.tensor_shape)
            dtype = mybir.dt.np(alloc.dtype)
            out_names.append(name)
            out_avals.append(jax.core.ShapedArray(shape, dtype))
            zero_shapes.append((shape, dtype))
    n_params = len(in_names)
    n_outs = len(out_avals)
    all_in_names = list(in_names) + list(out_names)
    if partition_name is not None:
        all_in_names.append(partition_name)

    def _body(*args):
        operands = list(args)
        if partition_name is not None:
            operands.append(bass2jax.partition_id_tensor())
        outs = bass2jax._bass_exec_p.bind(
            *operands,
            out_avals=tuple(out_avals),
            in_names=tuple(all_in_names),
            out_names=tuple(out_names),
            lowering_input_output_aliases=(),
            sim_require_finite=True,
            sim_require_nnan=True,
            nc=nc,
        )
        return tuple(outs)

    devices = jax.devices()[:N_CORES]
    mesh = Mesh(np.asarray(devices), ("core",))
    in_specs = (PartitionSpec("core"),) * (n_params + n_outs)
    out_specs = (PartitionSpec("core"),) * n_outs
    sharded = jax.jit(
        shard_map(
            _body, mesh=mesh, in_specs=in_specs, out_specs=out_specs, check_rep=False
        ),
        keep_unused=True,
    )

    from jax.sharding import NamedSharding

    zero_sharding = NamedSharding(mesh, PartitionSpec("core"))
    dev_zeros = [
        jax.device_put(np.zeros((N_CORES * s[0], *s[1:]), d), zero_sharding)
        for (s, d) in zero_shapes
    ]

    def run(in_maps):
        concat_in = [
            np.concatenate([np.asarray(m[name]) for m in in_maps], axis=0)
            for name in in_names
        ]
        out_arrs = sharded(*concat_in, *dev_zeros)
        return [
            {
                name: np.asarray(out_arrs[i]).reshape(
                    N_CORES, *out_avals[i].shape
                )[c]
                for i, name in enumerate(out_names)
            }
            for c in range(N_CORES)
        ]

    return nc, run


def _build_bands(k2):
    """k2: [B, 8, 8] fp32 -> Karatsuba bands [B, 128, 12, 121] bf16.

    h = q-reversed filter; H0/H1 = even/odd taps (4 each); planes
    0-3: Toeplitz bands of H0, 4-7: H1, 8-11: H0+H1.
    bands[b, m+p, plane(u), m] = Hx[b, p, u].
    """
    h = k2[:, :, ::-1]
    H0 = h[:, :, 0::2].astype(BF16)
    H1 = h[:, :, 1::2].astype(BF16)
    HS = (h[:, :, 0::2] + h[:, :, 1::2]).astype(BF16)
    bands = np.zeros((k2.shape[0], 128, 12, 121), BF16)
    m = np.arange(121)
    for p in range(KH):
        for u in range(4):
            bands[:, m + p, u, m] = H0[:, p, u][:, None]
            bands[:, m + p, 4 + u, m] = H1[:, p, u][:, None]
            bands[:, m + p, 8 + u, m] = HS[:, p, u][:, None]
    return bands


def _build_tailbands(k2):
    """k2: [N, 8, 8] -> block-diag tail bands [N//GS, GS*21, 8, GS*14]."""
    n = k2.shape[0]
    tb = np.zeros((n // GS, GS * TK, KW, GS * TM), BF16)
    m = np.arange(TM)
    k2 = k2.astype(BF16)
    for g in range(n // GS):
        for j in range(GS):
            for p in range(KH):
                tb[g, TK * j + m + p, :, TM * j + m] = k2[g * GS + j, p, :]
    return tb


def kernel(x, k):
    x = np.asarray(x, dtype=np.float32).reshape(B, H, W)
    k = np.asarray(k, dtype=np.float32).reshape(B, KH, KW)

    if "runner" not in _cache:
        _cache["runner"] = _build_runner()
    _nc, run = _cache["runner"]

    xs = x[:, :, 0::2] + x[:, :, 1::2]
    xb = np.concatenate([x, xs], axis=2).astype(BF16)
    bands = _build_bands(k)
    tailbands = _build_tailbands(k)
    n_groups = SPC // GS
    in_maps = [
        {
            "x": np.ascontiguousarray(xb[c * SPC : (c + 1) * SPC]),
            "bands": bands[c * SPC : (c + 1) * SPC],
            "tailbands": tailbands[c * n_groups : (c + 1) * n_groups],
        }
        for c in range(N_CORES)
    ]
    results = run(in_maps)
    out = np.concatenate([r["out"] for r in results], axis=0)
    return out.astype(np.float32).reshape(B, HO, WO, 1)


# revision 29
# speedup vs baseline: 1.9019x; 1.0017x over previous
"""Batched dynamic-filter cross-correlation on 8 Trainium2 NeuronCores.

Each sample b of x[128, 384, 384, 1] is VALID-correlated with its own
8x8 filter k[b] -> out[128, 377, 377, 1].

Strategy (pure data parallel, batch sharded 16 samples/core): the row
taps (p) contract on the TensorE partition dim via banded-Toeplitz
stationary matrices; the column taps (q) use a 2-parallel fast-FIR
(Karatsuba) decomposition that cuts TensorE streaming work to 3/4 of
direct: three 4-tap half-rate sub-correlations P0 = H0*X0, P2 = H1*X1,
P1 = (H0+H1)*(X0+X1), where X0/X1 are even/odd input columns (read
with stride-2 access patterns straight from the x tile) and H0/H1 are
even/odd taps of the q-reversed filter. Per 121-row output block this
is 12 PSUM-accumulating matmuls of N=189 instead of 8 of N=378.
Recombination (even = P1-P0-P2, odd[t] = P0[t+1]+P2[t]) runs on the
Activation engine (PSUM->SBUF stages of P0/P2) and the vector engine
(strided bf16 writes into the output tile).

Everything in HBM is bf16 (fp32 accumulation in PSUM; the host
upcasts): x ships as rows [x | X0+X1] (the pre-sum is computed on the
host and packed contiguously so DMA lines stay >= 512B), bands hold
the 12 Toeplitz tap-planes. DMA instruction count is minimized (one
x DMA per sample via an overlapping 3-window access pattern, one
banded-weights DMA per 4-sample group prefetched two samples early,
one output DMA per sample) because each DMA instruction serializes
~0.6-1us on the shared HWDGE/SWDGE descriptor generators; output DMAs
ride the otherwise-idle Pool SWDGE queue. The 14 leftover output rows
of 4 samples are packed into one block-diagonal 8-tap matmul group
(K=4*21, M=4*14) scheduled mid-group to stay off the critical path.
"""

import numpy as np
import ml_dtypes

BF16 = ml_dtypes.bfloat16
F8 = ml_dtypes.float8_e4m3


def _hi_lo(a):
    """fp32 array -> (hi, lo) fp8e4m3 pair with hi + lo ~= a."""
    hi = a.astype(F8)
    lo = (a - hi.astype(np.float32)).astype(F8)
    return hi, lo

B, H, W = 128, 384, 384
KH, KW = 8, 8
HO, WO = H - KH + 1, W - KW + 1          # 377, 377
N_CORES = 8
SPC = B // N_CORES                        # 16 samples per core

MAIN_BLOCKS = [(0, 121, 128), (121, 121, 128), (242, 121, 128)]
TB, TM, TK = 363, 14, 21                  # tail rows: out 363..376, in 363..383
GS = 4                                    # tail-group size (samples per group)
NO2 = WO + 1                              # 378: tail moving width
XW = 386                                  # tail x tile width (q=7 reads col 384)
XR = 576                                  # fp8 half-row: [x(384)|xs(192)]
NP = 189                                  # half-rate sub-conv output cols (m=3..191)
XH = 192                                  # half-rate input length

_cache = {}


def _build_program():
    import concourse.mybir as mybir
    import concourse.tile as tile
    from concourse import bacc

    from concourse.ap import AP

    bf16 = mybir.dt.bfloat16
    f8 = mybir.dt.float8e4
    f32 = mybir.dt.float32
    DR = mybir.MatmulPerfMode.DoubleRow

    def pair(ap2, stride):
        d = [list(p) for p in ap2.ap]
        assert len(d) == 2, d
        return AP(ap2.tensor, ap2.offset, [d[0], [stride, 2], d[1]])

    def win3(base, nrows, width, step):
        """[128, 3, width] view of a [H|HO, width] DRAM sample: row = step*t + kk."""
        return AP(base.tensor, base.offset,
                  [[width, nrows], [step * width, 3], [1, width]])

    nc = bacc.Bacc(None, target_bir_lowering=False)
    x_d = nc.dram_tensor("x", [SPC, H, 2 * XR], f8, kind="ExternalInput")
    b_d = nc.dram_tensor("bands", [SPC, 128, 24, 128], f8, kind="ExternalInput")
    t_d = nc.dram_tensor(
        "tailbands", [SPC // GS, GS * TK, 2 * KW, 128], f8, kind="ExternalInput"
    )
    o_d = nc.dram_tensor("out", [SPC, HO, WO], bf16, kind="ExternalOutput")

    with tile.TileContext(nc) as tc:
        with (
            tc.tile_pool(name="xp", bufs=3) as xp,
            tc.tile_pool(name="bp", bufs=2) as bp,
            tc.tile_pool(name="tbp", bufs=1) as tbp,
            tc.tile_pool(name="txp", bufs=2) as txp,
            tc.tile_pool(name="pa", bufs=2, space="PSUM") as pa,
            tc.tile_pool(name="pb", bufs=2, space="PSUM") as pb,
            tc.tile_pool(name="pc", bufs=2, space="PSUM") as pc,
            tc.tile_pool(name="pt", bufs=2, space="PSUM") as pt,
            tc.tile_pool(name="tp", bufs=6) as tp,
            tc.tile_pool(name="op", bufs=3) as op,
        ):
            # x prefetch rolls one sample ahead; bands are per-sample
            xts = {}
            xt0 = xp.tile([128, 3, 2 * XR], f8, name="xtn")
            xts[0] = xt0
            tt = None
            bts = {}
            for g in range(SPC // GS):
                if g == 0:
                    bta = bp.tile([128, 1, 24, 128], f8, name="bta")
                    nc.scalar.dma_start(out=bta[:, 0, 0:12], in_=b_d[0, :, 0:12])
                    nc.sync.dma_start(out=xt0[:, 0], in_=x_d[0, 0:128])
                    nc.scalar.dma_start(out=bta[:, 0, 12:24], in_=b_d[0, :, 12:24])
                    nc.sync.dma_start(out=xt0[:, 1], in_=x_d[0, 121:249])
                    nc.sync.dma_start(out=xt0[:, 2], in_=x_d[0, 242:370])
                    bt = bp.tile([128, GS, 24, 128], f8, name="bt")
                    bts[0] = bt
                    nc.sync.dma_start(
                        out=bt[:, 1:],
                        in_=b_d[1:GS].transpose([1, 0, 2, 3]),
                    )
                bt = bts.pop(g)
                xtt = txp.tile([GS * TK, 2 * XR], f8)
                nc.sync.dma_start(
                    out=xtt[:], in_=x_d[g * GS : (g + 1) * GS, TB : TB + TK, :]
                )
                for j in range(GS):
                    s = g * GS + j
                    if s + 1 < SPC:
                        xtn = xp.tile([128, 3, 2 * XR], f8, name="xtn")
                        xts[s + 1] = xtn
                        nc.sync.dma_start(
                            out=xtn[:], in_=win3(x_d[s + 1], 128, 2 * XR, 121)
                        )
                    if tt is None:
                        tt = tbp.tile([GS * TK, SPC // GS, 2 * KW, 128], f8)
                        nc.sync.dma_start(out=tt[:], in_=t_d[:].transpose([1, 0, 2, 3]))
                    xt = xts.pop(s)
                    btj = bta[:, 0] if s == 0 else bt[:, j]
                    ot = op.tile([121, 3, WO], bf16)
                    for bi, (obase, M, K) in enumerate(MAIN_BLOCKS):
                        ps0 = pa.tile([128, 512], f32)
                        ps2 = pb.tile([128, 512], f32)
                        ps1 = pc.tile([128, 512], f32)
                        for sc, psx in ((0, ps0), (1, ps2), (2, ps1)):
                            if sc == 0:
                                offs = [2 * (3 - u) for u in range(4)]
                                step = 2
                            elif sc == 1:
                                offs = [2 * (3 - u) + 1 for u in range(4)]
                                step = 2
                            else:
                                offs = [W + 3 - u for u in range(4)]
                                step = 1
                            for u in range(4):
                                xhi = xt[
                                    :K, bi,
                                    offs[u] : offs[u] + step * NP - step + 1 : step,
                                ]
                                nc.tensor.matmul(
                                    psx[:, :NP],
                                    pair(btj[:K, 4 * sc + u, :], 0),
                                    pair(xhi, XR),
                                    start=(u == 0),
                                    stop=False,
                                    perf_mode=DR,
                                )
                            for pi, (u0, u1) in enumerate(((0, 2), (1, 3))):
                                xhi = xt[
                                    :K, bi,
                                    offs[u0] : offs[u0] + step * NP - step + 1 : step,
                                ]
                                nc.tensor.matmul(
                                    psx[:, :NP],
                                    pair(btj[:K, 12 + 4 * sc + u0, :], 256),
                                    pair(xhi, offs[u1] - offs[u0]),
                                    start=False,
                                    stop=(pi == 1),
                                    perf_mode=DR,
                                )
                        # stage P0/P2 in SBUF (Act), recombine on DVE:
                        # even = P1-P0-P2, odd[t] = P0[t+1]+P2[t]
                        c0 = tp.tile([128, NP], f32)
                        nc.scalar.copy(out=c0[:M, :], in_=ps0[:M, :NP])
                        c2 = tp.tile([128, NP], f32)
                        nc.scalar.copy(out=c2[:M, :], in_=ps2[:M, :NP])
                        nc.vector.tensor_add(
                            out=ot[:M, bi, 1 : WO : 2],
                            in0=c0[:M, 1:NP],
                            in1=c2[:M, 0 : NP - 1],
                        )
                        t0 = tp.tile([128, NP], f32)
                        nc.vector.tensor_sub(
                            out=t0[:M, :], in0=ps1[:M, :NP], in1=c0[:M, :]
                        )
                        nc.vector.tensor_sub(
                            out=ot[:M, bi, 0 : WO : 2], in0=t0[:M, :], in1=c2[:M, :]
                        )
                    if s == SPC - 1:
                        for bi2, (ob2, M2, _K2) in enumerate(MAIN_BLOCKS):
                            nc.sync.dma_start(
                                out=o_d[s, ob2 : ob2 + M2, :],
                                in_=ot[:M2, bi2, :],
                            )
                    else:
                        nc.gpsimd.dma_start(
                            out=win3(o_d[s], 121, WO, 121), in_=ot[:]
                        )
                    if j == 2 and g + 1 < SPC // GS:
                        g2 = g + 1
                        btn = bp.tile([128, GS, 24, 128], f8, name="bt")
                        bts[g2] = btn
                        nc.sync.dma_start(
                            out=btn[:],
                            in_=b_d[g2 * GS : (g2 + 1) * GS].transpose([1, 0, 2, 3]),
                        )
                    if j == 1:
                        # tail: GS samples' last 14 rows, block-diagonal matmul
                        ps = pt.tile([128, 512], f32)
                        for q in range(KW):
                            nc.tensor.matmul(
                                ps[:, :WO],
                                pair(tt[: GS * TK, g, q, :], 0),
                                pair(xtt[: GS * TK, q : q + WO], XR),
                                start=(q == 0),
                                stop=False,
                                perf_mode=DR,
                            )
                        for pi, (q0, q1) in enumerate(((0, 2), (1, 3), (4, 6), (5, 7))):
                            nc.tensor.matmul(
                                ps[:, :WO],
                                pair(tt[: GS * TK, g, KW + q0, :], 256),
                                pair(xtt[: GS * TK, q0 : q0 + WO], q1 - q0),
                                start=False,
                                stop=(pi == 3),
                                perf_mode=DR,
                            )
                        oto = op.tile([121, 3, WO], bf16)
                        nc.scalar.copy(
                            out=oto[: GS * TM, 0, :], in_=ps[: GS * TM, :WO]
                        )
                        nc.gpsimd.dma_start(
                            out=o_d[g * GS : (g + 1) * GS, TB : TB + TM, :],
                            in_=oto[: GS * TM, 0, :],
                        )

    nc.compile()
    return nc


def _build_runner():
    """Build nc + a persistent jitted PJRT callable (compiles once)."""
    import jax
    from jax.sharding import Mesh, PartitionSpec
    from jax.experimental.shard_map import shard_map
    import concourse.mybir as mybir
    from concourse import bass2jax

    nc = _build_program()
    bass2jax.install_neuronx_cc_hook()

    partition_name = nc.partition_id_tensor.name if nc.partition_id_tensor else None

    in_names, out_names, out_avals, zero_shapes = [], [], [], []
    for alloc in nc.m.functions[0].allocations:
        if not isinstance(alloc, mybir.MemoryLocationSet):
            continue
        name = alloc.memorylocations[0].name
        if alloc.kind == "ExternalInput":
            if name != partition_name:
                in_names.append(name)
        elif alloc.kind == "ExternalOutput":
            shape = tuple(alloc.tensor_shape)
            dtype = mybir.dt.np(alloc.dtype)
            out_names.append(name)
            out_avals.append(jax.core.ShapedArray(shape, dtype))
            zero_shapes.append((shape, dtype))
    n_params = len(in_names)
    n_outs = len(out_avals)
    all_in_names = list(in_names) + list(out_names)
    if partition_name is not None:
        all_in_names.append(partition_name)

    def _body(*args):
        operands = list(args)
        if partition_name is not None:
            operands.append(bass2jax.partition_id_tensor())
        outs = bass2jax._bass_exec_p.bind(
            *operands,
            out_avals=tuple(out_avals),
            in_names=tuple(all_in_names),
            out_names=tuple(out_names),
            lowering_input_output_aliases=(),
            sim_require_finite=True,
            sim_require_nnan=True,
            nc=nc,
        )
        return tuple(outs)

    devices = jax.devices()[:N_CORES]
    mesh = Mesh(np.asarray(devices), ("core",))
    in_specs = (PartitionSpec("core"),) * (n_params + n_outs)
    out_specs = (PartitionSpec("core"),) * n_outs
    sharded = jax.jit(
        shard_map(
            _body, mesh=mesh, in_specs=in_specs, out_specs=out_specs, check_rep=False
        ),
        keep_unused=True,
    )

    from jax.sharding import NamedSharding

    zero_sharding = NamedSharding(mesh, PartitionSpec("core"))
    dev_zeros = [
        jax.device_put(np.zeros((N_CORES * s[0], *s[1:]), d), zero_sharding)
        for (s, d) in zero_shapes
    ]

    def run(in_maps):
        concat_in = [
            np.concatenate([np.asarray(m[name]) for m in in_maps], axis=0)
            for name in in_names
        ]
        out_arrs = sharded(*concat_in, *dev_zeros)
        return [
            {
                name: np.asarray(out_arrs[i]).reshape(
                    N_CORES, *out_avals[i].shape
                )[c]
                for i, name in enumerate(out_names)
            }
            for c in range(N_CORES)
        ]

    return nc, run


def _build_bands(k2):
    """k2: [B, 8, 8] fp32 -> fp8 hi/lo Karatsuba bands [B, 128, 24, 121].

    fp32 planes 0-3: Toeplitz bands of H0 (even taps of q-reversed k),
    4-7: H1, 8-11: H0+H1; fp8 planes 0-11 = hi split, 12-23 = lo
    residual. bands[b, m+p, plane(u), m] = Hx[b, p, u].
    """
    h = k2[:, :, ::-1]
    H = np.concatenate([h[:, :, 0::2], h[:, :, 1::2],
                        h[:, :, 0::2] + h[:, :, 1::2]], axis=2)  # [B, 8, 12]
    bands = np.zeros((k2.shape[0], 128, 12, 128), np.float32)
    m = np.arange(121)
    for p in range(KH):
        for u in range(12):
            bands[:, m + p, u, m] = H[:, p, u][:, None]
    hi, lo = _hi_lo(bands)
    return np.concatenate([hi, lo], axis=2)


def _build_tailbands(k2):
    """k2: [N, 8, 8] -> fp8 hi/lo block-diag tail bands [N//GS, GS*21, 16, GS*14]."""
    n = k2.shape[0]
    tb = np.zeros((n // GS, GS * TK, KW, 128), np.float32)
    m = np.arange(TM)
    for g in range(n // GS):
        for j in range(GS):
            for p in range(KH):
                tb[g, TK * j + m + p, :, TM * j + m] = k2[g * GS + j, p, :]
    hi, lo = _hi_lo(tb)
    return np.concatenate([hi, lo], axis=2)


def kernel(x, k):
    x = np.asarray(x, dtype=np.float32).reshape(B, H, W)
    k = np.asarray(k, dtype=np.float32).reshape(B, KH, KW)

    if "runner" not in _cache:
        _cache["runner"] = _build_runner()
    _nc, run = _cache["runner"]

    xs = x[:, :, 0::2] + x[:, :, 1::2]
    row = np.concatenate([x, xs], axis=2)
    xh, xl = _hi_lo(row)
    xb = np.concatenate([xh, xl], axis=2)
    bands = _build_bands(k)
    tailbands = _build_tailbands(k)
    n_groups = SPC // GS
    in_maps = [
        {
            "x": np.ascontiguousarray(xb[c * SPC : (c + 1) * SPC]),
            "bands": bands[c * SPC : (c + 1) * SPC],
            "tailbands": tailbands[c * n_groups : (c + 1) * n_groups],
        }
        for c in range(N_CORES)
    ]
    results = run(in_maps)
    out = np.concatenate([r["out"] for r in results], axis=0)
    return out.astype(np.float32).reshape(B, HO, WO, 1)


# revision 33
# speedup vs baseline: 1.9135x; 1.0061x over previous
"""Batched dynamic-filter cross-correlation on 8 Trainium2 NeuronCores.

Each sample b of x[128, 384, 384, 1] is VALID-correlated with its own
8x8 filter k[b] -> out[128, 377, 377, 1].

Strategy (pure data parallel, batch sharded 16 samples/core): the row
taps (p) contract on the TensorE partition dim via banded-Toeplitz
stationary matrices; the column taps (q) use a 2-parallel fast-FIR
(Karatsuba) decomposition that cuts TensorE streaming work to 3/4 of
direct: three 4-tap half-rate sub-correlations P0 = H0*X0, P2 = H1*X1,
P1 = (H0+H1)*(X0+X1), where X0/X1 are even/odd input columns (read
with stride-2 access patterns straight from the x tile) and H0/H1 are
even/odd taps of the q-reversed filter. Per 121-row output block this
is 12 PSUM-accumulating matmuls of N=189 instead of 8 of N=378.
Recombination (even = P1-P0-P2, odd[t] = P0[t+1]+P2[t]) runs on the
Activation engine (PSUM->SBUF stages of P0/P2) and the vector engine
(strided bf16 writes into the output tile).

Operands ship as fp8e4m3 hi/lo pairs (hi + lo ~= fp32 value, ~7-bit
effective mantissa) and every matmul is an fp8 DoubleRow instruction:
two (weights, ifmap) K-tiles contract per instruction at 0.5
cycles/row, so the 3-term compensated product set (hi*hi + lo*hi +
hi*lo) costs 0.75x the bf16 streaming work. DoubleRow LdWeights
requires exactly 128 weight columns, so band planes are padded to
M=128 (output rows 121..127 compute zeros and are ignored). x ships
as rows [hi(x|X0+X1) | lo(x|X0+X1)] packed contiguously so DMA lines
stay >= 512B; bands hold 12 hi + 12 lo Toeplitz tap-planes. PSUM
accumulates fp32; the output tensor is bf16 and the host upcasts. DMA instruction count is minimized (one
x DMA per sample via an overlapping 3-window access pattern, one
banded-weights DMA per 4-sample group prefetched two samples early,
one output DMA per sample) because each DMA instruction serializes
~0.6-1us on the shared HWDGE/SWDGE descriptor generators; output DMAs
ride the otherwise-idle Pool SWDGE queue. The 14 leftover output rows
of 4 samples are packed into one block-diagonal 8-tap matmul group
(K=4*21, M=4*14) scheduled mid-group to stay off the critical path.
"""

import numpy as np
import ml_dtypes

BF16 = ml_dtypes.bfloat16
F8 = ml_dtypes.float8_e4m3


def _hi_lo(a):
    """fp32 array -> (hi, lo) fp8e4m3 pair with hi + lo ~= a."""
    hi = a.astype(F8)
    lo = (a - hi.astype(np.float32)).astype(F8)
    return hi, lo

B, H, W = 128, 384, 384
KH, KW = 8, 8
HO, WO = H - KH + 1, W - KW + 1          # 377, 377
N_CORES = 8
SPC = B // N_CORES                        # 16 samples per core

MAIN_BLOCKS = [(0, 121, 128), (121, 121, 128), (242, 121, 128)]
TB, TM, TK = 363, 14, 21                  # tail rows: out 363..376, in 363..383
GS = 4                                    # tail-group size (samples per group)
NO2 = WO + 1                              # 378: tail moving width
XW = 386                                  # tail x tile width (q=7 reads col 384)
XR = 576                                  # fp8 half-row: [x(384)|xs(192)]
NP = 189                                  # half-rate sub-conv output cols (m=3..191)
XH = 192                                  # half-rate input length

_cache = {}


def _build_program():
    import concourse.mybir as mybir
    import concourse.tile as tile
    from concourse import bacc

    from concourse.ap import AP

    bf16 = mybir.dt.bfloat16
    f8 = mybir.dt.float8e4
    f32 = mybir.dt.float32
    DR = mybir.MatmulPerfMode.DoubleRow

    def pair(ap2, stride):
        d = [list(p) for p in ap2.ap]
        assert len(d) == 2, d
        return AP(ap2.tensor, ap2.offset, [d[0], [stride, 2], d[1]])

    def win3(base, nrows, width, step):
        """[128, 3, width] view of a [H|HO, width] DRAM sample: row = step*t + kk."""
        return AP(base.tensor, base.offset,
                  [[width, nrows], [step * width, 3], [1, width]])

    nc = bacc.Bacc(None, target_bir_lowering=False)
    x_d = nc.dram_tensor("x", [SPC, H, 2 * XR], f8, kind="ExternalInput")
    b_d = nc.dram_tensor("bands", [SPC, 128, 24, 128], f8, kind="ExternalInput")
    t_d = nc.dram_tensor(
        "tailbands", [SPC // GS, GS * TK, 2 * KW, 128], f8, kind="ExternalInput"
    )
    o_d = nc.dram_tensor("out", [SPC, HO, WO], bf16, kind="ExternalOutput")

    with tile.TileContext(nc) as tc:
        with (
            tc.tile_pool(name="xp", bufs=6) as xp,
            tc.tile_pool(name="bp", bufs=3) as bp,
            tc.tile_pool(name="tbp", bufs=1) as tbp,
            tc.tile_pool(name="txp", bufs=2) as txp,
            tc.tile_pool(name="pa", bufs=2, space="PSUM") as pa,
            tc.tile_pool(name="pb", bufs=2, space="PSUM") as pb,
            tc.tile_pool(name="pc", bufs=2, space="PSUM") as pc,
            tc.tile_pool(name="pt", bufs=2, space="PSUM") as pt,
            tc.tile_pool(name="tp", bufs=6) as tp,
            tc.tile_pool(name="op", bufs=5) as op,
        ):
            # x prefetch rolls one sample ahead; bands are per-sample
            xts = {}
            xt0 = xp.tile([128, 3, 2 * XR], f8, name="xtn")
            xts[0] = xt0
            tt = None
            bts = {}
            for g in range(SPC // GS):
                if g == 0:
                    bta = bp.tile([128, 1, 24, 128], f8, name="bta")
                    nc.scalar.dma_start(out=bta[:, 0, 0:12], in_=b_d[0, :, 0:12])
                    nc.sync.dma_start(out=xt0[:, 0], in_=x_d[0, 0:128])
                    nc.scalar.dma_start(out=bta[:, 0, 12:24], in_=b_d[0, :, 12:24])
                    nc.sync.dma_start(out=xt0[:, 1], in_=x_d[0, 121:249])
                    nc.sync.dma_start(out=xt0[:, 2], in_=x_d[0, 242:370])
                    bt = bp.tile([128, GS, 24, 128], f8, name="bt")
                    bts[0] = bt
                    nc.sync.dma_start(
                        out=bt[:, 1:],
                        in_=b_d[1:GS].transpose([1, 0, 2, 3]),
                    )
                    for sp_ in (1, 2):
                        xtn = xp.tile([128, 3, 2 * XR], f8, name="xtn")
                        xts[sp_] = xtn
                        nc.sync.dma_start(
                            out=xtn[:], in_=win3(x_d[sp_], 128, 2 * XR, 121)
                        )
                bt = bts.pop(g)
                xtt = txp.tile([GS * TK, 2 * XR], f8)
                nc.sync.dma_start(
                    out=xtt[:], in_=x_d[g * GS : (g + 1) * GS, TB : TB + TK, :]
                )
                for j in range(GS):
                    s = g * GS + j
                    if s + 3 < SPC:
                        xtn = xp.tile([128, 3, 2 * XR], f8, name="xtn")
                        xts[s + 3] = xtn
                        nc.sync.dma_start(
                            out=xtn[:], in_=win3(x_d[s + 3], 128, 2 * XR, 121)
                        )
                    if tt is None:
                        tt = tbp.tile([GS * TK, SPC // GS, 2 * KW, 128], f8)
                        nc.sync.dma_start(out=tt[:], in_=t_d[:].transpose([1, 0, 2, 3]))
                    xt = xts.pop(s)
                    btj = bta[:, 0] if s == 0 else bt[:, j]
                    ot = op.tile([121, 3, WO], bf16)
                    for bi, (obase, M, K) in enumerate(MAIN_BLOCKS):
                        ps0 = pa.tile([128, 512], f32)
                        ps2 = pb.tile([128, 512], f32)
                        ps1 = pc.tile([128, 512], f32)
                        for sc, psx in ((0, ps0), (1, ps2), (2, ps1)):
                            if sc == 0:
                                offs = [2 * (3 - u) for u in range(4)]
                                step = 2
                            elif sc == 1:
                                offs = [2 * (3 - u) + 1 for u in range(4)]
                                step = 2
                            else:
                                offs = [W + 3 - u for u in range(4)]
                                step = 1
                            for u in range(4):
                                xhi = xt[
                                    :K, bi,
                                    offs[u] : offs[u] + step * NP - step + 1 : step,
                                ]
                                nc.tensor.matmul(
                                    psx[:, :NP],
                                    pair(btj[:K, 4 * sc + u, :], 0),
                                    pair(xhi, XR),
                                    start=(u == 0),
                                    stop=False,
                                    perf_mode=DR,
                                )
                            for pi, (u0, u1) in enumerate(((0, 2), (1, 3))):
                                xhi = xt[
                                    :K, bi,
                                    offs[u0] : offs[u0] + step * NP - step + 1 : step,
                                ]
                                nc.tensor.matmul(
                                    psx[:, :NP],
                                    pair(btj[:K, 12 + 4 * sc + u0, :], 256),
                                    pair(xhi, offs[u1] - offs[u0]),
                                    start=False,
                                    stop=(pi == 1),
                                    perf_mode=DR,
                                )
                        # stage P0/P2 in SBUF (Act), recombine on DVE:
                        # even = P1-P0-P2, odd[t] = P0[t+1]+P2[t]
                        c0 = tp.tile([128, NP], f32)
                        nc.scalar.copy(out=c0[:M, :], in_=ps0[:M, :NP])
                        c2 = tp.tile([128, NP], f32)
                        nc.scalar.copy(out=c2[:M, :], in_=ps2[:M, :NP])
                        nc.vector.tensor_add(
                            out=ot[:M, bi, 1 : WO : 2],
                            in0=c0[:M, 1:NP],
                            in1=c2[:M, 0 : NP - 1],
                        )
                        t0 = tp.tile([128, NP], f32)
                        nc.vector.tensor_sub(
                            out=t0[:M, :], in0=ps1[:M, :NP], in1=c0[:M, :]
                        )
                        nc.vector.tensor_sub(
                            out=ot[:M, bi, 0 : WO : 2], in0=t0[:M, :], in1=c2[:M, :]
                        )
                    if s == SPC - 1:
                        for bi2, (ob2, M2, _K2) in enumerate(MAIN_BLOCKS):
                            nc.sync.dma_start(
                                out=o_d[s, ob2 : ob2 + M2, :],
                                in_=ot[:M2, bi2, :],
                            )
                    else:
                        nc.gpsimd.dma_start(
                            out=win3(o_d[s], 121, WO, 121), in_=ot[:]
                        )
                    if j == 0 and g + 1 < SPC // GS:
                        g2 = g + 1
                        btn = bp.tile([128, GS, 24, 128], f8, name="bt")
                        bts[g2] = btn
                        nc.sync.dma_start(
                            out=btn[:],
                            in_=b_d[g2 * GS : (g2 + 1) * GS].transpose([1, 0, 2, 3]),
                        )
                    if j == 1:
                        # tail: GS samples' last 14 rows, block-diagonal matmul
                        ps = pt.tile([128, 512], f32)
                        for q in range(KW):
                            nc.tensor.matmul(
                                ps[:, :WO],
                                pair(tt[: GS * TK, g, q, :], 0),
                                pair(xtt[: GS * TK, q : q + WO], XR),
                                start=(q == 0),
                                stop=False,
                                perf_mode=DR,
                            )
                        for pi, (q0, q1) in enumerate(((0, 2), (1, 3), (4, 6), (5, 7))):
                            nc.tensor.matmul(
                                ps[:, :WO],
                                pair(tt[: GS * TK, g, KW + q0, :], 256),
                                pair(xtt[: GS * TK, q0 : q0 + WO], q1 - q0),
                                start=False,
                                stop=(pi == 3),
                                perf_mode=DR,
                            )
                        oto = op.tile([121, 3, WO], bf16)
                        nc.scalar.copy(
                            out=oto[: GS * TM, 0, :], in_=ps[: GS * TM, :WO]
                        )
                        nc.gpsimd.dma_start(
                            out=o_d[g * GS : (g + 1) * GS, TB : TB + TM, :],
                            in_=oto[: GS * TM, 0, :],
                        )

    nc.compile()
    return nc


def _build_runner():
    """Build nc + a persistent jitted PJRT callable (compiles once)."""
    import jax
    from jax.sharding import Mesh, PartitionSpec
    from jax.experimental.shard_map import shard_map
    import concourse.mybir as mybir
    from concourse import bass2jax

    nc = _build_program()
    bass2jax.install_neuronx_cc_hook()

    partition_name = nc.partition_id_tensor.name if nc.partition_id_tensor else None

    in_names, out_names, out_avals, zero_shapes = [], [], [], []
    for alloc in nc.m.functions[0].allocations:
        if not isinstance(alloc, mybir.MemoryLocationSet):
            continue
        name = alloc.memorylocations[0].name
        if alloc.kind == "ExternalInput":
            if name != partition_name:
                in_names.append(name)
        elif alloc.kind == "ExternalOutput":
            shape = tuple(alloc.tensor_shape)
            dtype = mybir.dt.np(alloc.dtype)
            out_names.append(name)
            out_avals.append(jax.core.ShapedArray(shape, dtype))
            zero_shapes.append((shape, dtype))
    n_params = len(in_names)
    n_outs = len(out_avals)
    all_in_names = list(in_names) + list(out_names)
    if partition_name is not None:
        all_in_names.append(partition_name)

    def _body(*args):
        operands = list(args)
        if partition_name is not None:
            operands.append(bass2jax.partition_id_tensor())
        outs = bass2jax._bass_exec_p.bind(
            *operands,
            out_avals=tuple(out_avals),
            in_names=tuple(all_in_names),
            out_names=tuple(out_names),
            lowering_input_output_aliases=(),
            sim_require_finite=True,
            sim_require_nnan=True,
            nc=nc,
        )
        return tuple(outs)

    devices = jax.devices()[:N_CORES]
    mesh = Mesh(np.asarray(devices), ("core",))
    in_specs = (PartitionSpec("core"),) * (n_params + n_outs)
    out_specs = (PartitionSpec("core"),) * n_outs
    sharded = jax.jit(
        shard_map(
            _body, mesh=mesh, in_specs=in_specs, out_specs=out_specs, check_rep=False
        ),
        keep_unused=True,
    )

    from jax.sharding import NamedSharding

    zero_sharding = NamedSharding(mesh, PartitionSpec("core"))
    dev_zeros = [
        jax.device_put(np.zeros((N_CORES * s[0], *s[1:]), d), zero_sharding)
        for (s, d) in zero_shapes
    ]

    def run(in_maps):
        concat_in = [
            np.concatenate([np.asarray(m[name]) for m in in_maps], axis=0)
            for name in in_names
        ]
        out_arrs = sharded(*concat_in, *dev_zeros)
        return [
            {
                name: np.asarray(out_arrs[i]).reshape(
                    N_CORES, *out_avals[i].shape
                )[c]
                for i, name in enumerate(out_names)
            }
            for c in range(N_CORES)
        ]

    return nc, run


def _build_bands(k2):
    """k2: [B, 8, 8] fp32 -> fp8 hi/lo Karatsuba bands [B, 128, 24, 121].

    fp32 planes 0-3: Toeplitz bands of H0 (even taps of q-reversed k),
    4-7: H1, 8-11: H0+H1; fp8 planes 0-11 = hi split, 12-23 = lo
    residual. bands[b, m+p, plane(u), m] = Hx[b, p, u].
    """
    h = k2[:, :, ::-1]
    H = np.concatenate([h[:, :, 0::2], h[:, :, 1::2],
                        h[:, :, 0::2] + h[:, :, 1::2]], axis=2)  # [B, 8, 12]
    bands = np.zeros((k2.shape[0], 128, 12, 128), np.float32)
    m = np.arange(121)
    for p in range(KH):
        for u in range(12):
            bands[:, m + p, u, m] = H[:, p, u][:, None]
    hi, lo = _hi_lo(bands)
    return np.concatenate([hi, lo], axis=2)


def _build_tailbands(k2):
    """k2: [N, 8, 8] -> fp8 hi/lo block-diag tail bands [N//GS, GS*21, 16, GS*14]."""
    n = k2.shape[0]
    tb = np.zeros((n // GS, GS * TK, KW, 128), np.float32)
    m = np.arange(TM)
    for g in range(n // GS):
        for j in range(GS):
            for p in range(KH):
                tb[g, TK * j + m + p, :, TM * j + m] = k2[g * GS + j, p, :]
    hi, lo = _hi_lo(tb)
    return np.concatenate([hi, lo], axis=2)


def kernel(x, k):
    x = np.asarray(x, dtype=np.float32).reshape(B, H, W)
    k = np.asarray(k, dtype=np.float32).reshape(B, KH, KW)

    if "runner" not in _cache:
        _cache["runner"] = _build_runner()
    _nc, run = _cache["runner"]

    xs = x[:, :, 0::2] + x[:, :, 1::2]
    row = np.concatenate([x, xs], axis=2)
    xh, xl = _hi_lo(row)
    xb = np.concatenate([xh, xl], axis=2)
    bands = _build_bands(k)
    tailbands = _build_tailbands(k)
    n_groups = SPC // GS
    in_maps = [
        {
            "x": np.ascontiguousarray(xb[c * SPC : (c + 1) * SPC]),
            "bands": bands[c * SPC : (c + 1) * SPC],
            "tailbands": tailbands[c * n_groups : (c + 1) * n_groups],
        }
        for c in range(N_CORES)
    ]
    results = run(in_maps)
    out = np.concatenate([r["out"] for r in results], axis=0)
    return out.astype(np.float32).reshape(B, HO, WO, 1)


# revision 35
# speedup vs baseline: 1.9973x; 1.0438x over previous
"""Batched dynamic-filter cross-correlation on 8 Trainium2 NeuronCores.

Each sample b of x[128, 384, 384, 1] is VALID-correlated with its own
8x8 filter k[b] -> out[128, 377, 377, 1].

Strategy (pure data parallel, batch sharded 16 samples/core): the row
taps (p) contract on the TensorE partition dim via banded-Toeplitz
stationary matrices; the column taps (q) use a 2-parallel fast-FIR
(Karatsuba) decomposition that cuts TensorE streaming work to 3/4 of
direct: three 4-tap half-rate sub-correlations P0 = H0*X0, P2 = H1*X1,
P1 = (H0+H1)*(X0+X1), where X0/X1 are even/odd input columns (read
with stride-2 access patterns straight from the x tile) and H0/H1 are
even/odd taps of the q-reversed filter. Per 121-row output block this
is 12 PSUM-accumulating matmuls of N=189 instead of 8 of N=378.
Recombination (even = P1-P0-P2, odd[t] = P0[t+1]+P2[t]) runs on the
Activation engine (PSUM->SBUF stages of P0/P2) and the vector engine
(strided bf16 writes into the output tile).

Operands ship as fp8e4m3 hi/lo pairs (hi + lo ~= fp32 value, ~7-bit
effective mantissa) and every matmul is an fp8 DoubleRow instruction:
two (weights, ifmap) K-tiles contract per instruction at 0.5
cycles/row, so the 3-term compensated product set (hi*hi + lo*hi +
hi*lo) costs 0.75x the bf16 streaming work. DoubleRow LdWeights
requires exactly 128 weight columns, so band planes are padded to
M=128 (output rows 121..127 compute zeros and are ignored). x ships
as rows [hi(x|X0+X1) | lo(x|X0+X1)] packed contiguously so DMA lines
stay >= 512B; bands hold 8 hi + 8 lo Toeplitz tap-planes (H0/H1 only
— P1's H0+H1 weights are summed on the fly by DoubleRow weight pairs
with a stride-0 ifmap pair, trading 6 extra PE instructions per block
for 2.1 MB less band traffic). PSUM
accumulates fp32; the output tensor is bf16 and the host upcasts. DMA instruction count is minimized (one
x DMA per sample via an overlapping 3-window access pattern, one
banded-weights DMA per 4-sample group prefetched two samples early,
one output DMA per sample) because each DMA instruction serializes
~0.6-1us on the shared HWDGE/SWDGE descriptor generators; output DMAs
ride the otherwise-idle Pool SWDGE queue. The 14 leftover output rows
of 4 samples are packed into one block-diagonal 8-tap matmul group
(K=4*21, M=4*14) scheduled mid-group to stay off the critical path.
"""

import numpy as np
import ml_dtypes

BF16 = ml_dtypes.bfloat16
F8 = ml_dtypes.float8_e4m3


def _hi_lo(a):
    """fp32 array -> (hi, lo) fp8e4m3 pair with hi + lo ~= a."""
    hi = a.astype(F8)
    lo = (a - hi.astype(np.float32)).astype(F8)
    return hi, lo

B, H, W = 128, 384, 384
KH, KW = 8, 8
HO, WO = H - KH + 1, W - KW + 1          # 377, 377
N_CORES = 8
SPC = B // N_CORES                        # 16 samples per core

MAIN_BLOCKS = [(0, 121, 128), (121, 121, 128), (242, 121, 128)]
TB, TM, TK = 363, 14, 21                  # tail rows: out 363..376, in 363..383
GS = 4                                    # tail-group size (samples per group)
NO2 = WO + 1                              # 378: tail moving width
XW = 386                                  # tail x tile width (q=7 reads col 384)
XR = 576                                  # fp8 half-row: [x(384)|xs(192)]
NP = 189                                  # half-rate sub-conv output cols (m=3..191)
XH = 192                                  # half-rate input length

_cache = {}


def _build_program():
    import concourse.mybir as mybir
    import concourse.tile as tile
    from concourse import bacc

    from concourse.ap import AP

    bf16 = mybir.dt.bfloat16
    f8 = mybir.dt.float8e4
    f32 = mybir.dt.float32
    DR = mybir.MatmulPerfMode.DoubleRow

    def pair(ap2, stride):
        d = [list(p) for p in ap2.ap]
        assert len(d) == 2, d
        return AP(ap2.tensor, ap2.offset, [d[0], [stride, 2], d[1]])

    def win3(base, nrows, width, step):
        """[128, 3, width] view of a [H|HO, width] DRAM sample: row = step*t + kk."""
        return AP(base.tensor, base.offset,
                  [[width, nrows], [step * width, 3], [1, width]])

    nc = bacc.Bacc(None, target_bir_lowering=False)
    x_d = nc.dram_tensor("x", [SPC, H, 2 * XR], f8, kind="ExternalInput")
    b_d = nc.dram_tensor("bands", [SPC, 128, 16, 128], f8, kind="ExternalInput")
    t_d = nc.dram_tensor(
        "tailbands", [SPC // GS, GS * TK, 2 * KW, 128], f8, kind="ExternalInput"
    )
    o_d = nc.dram_tensor("out", [SPC, HO, WO], bf16, kind="ExternalOutput")

    with tile.TileContext(nc) as tc:
        with (
            tc.tile_pool(name="xp", bufs=6) as xp,
            tc.tile_pool(name="bp", bufs=3) as bp,
            tc.tile_pool(name="tbp", bufs=1) as tbp,
            tc.tile_pool(name="txp", bufs=2) as txp,
            tc.tile_pool(name="pa", bufs=2, space="PSUM") as pa,
            tc.tile_pool(name="pb", bufs=2, space="PSUM") as pb,
            tc.tile_pool(name="pc", bufs=2, space="PSUM") as pc,
            tc.tile_pool(name="pt", bufs=2, space="PSUM") as pt,
            tc.tile_pool(name="tp", bufs=6) as tp,
            tc.tile_pool(name="op", bufs=5) as op,
        ):
            # x prefetch rolls one sample ahead; bands are per-sample
            xts = {}
            xt0 = xp.tile([128, 3, 2 * XR], f8, name="xtn")
            xts[0] = xt0
            tt = None
            bts = {}
            for g in range(SPC // GS):
                if g == 0:
                    bta = bp.tile([128, 1, 16, 128], f8, name="bta")
                    nc.scalar.dma_start(out=bta[:, 0, 0:8], in_=b_d[0, :, 0:8])
                    nc.sync.dma_start(out=xt0[:, 0], in_=x_d[0, 0:128])
                    nc.scalar.dma_start(out=bta[:, 0, 8:16], in_=b_d[0, :, 8:16])
                    nc.sync.dma_start(out=xt0[:, 1], in_=x_d[0, 121:249])
                    nc.sync.dma_start(out=xt0[:, 2], in_=x_d[0, 242:370])
                    bt = bp.tile([128, GS, 16, 128], f8, name="bt")
                    bts[0] = bt
                    nc.sync.dma_start(
                        out=bt[:, 1:],
                        in_=b_d[1:GS].transpose([1, 0, 2, 3]),
                    )
                    for sp_ in (1, 2):
                        xtn = xp.tile([128, 3, 2 * XR], f8, name="xtn")
                        xts[sp_] = xtn
                        nc.sync.dma_start(
                            out=xtn[:], in_=win3(x_d[sp_], 128, 2 * XR, 121)
                        )
                bt = bts.pop(g)
                xtt = txp.tile([GS * TK, 2 * XR], f8)
                nc.sync.dma_start(
                    out=xtt[:], in_=x_d[g * GS : (g + 1) * GS, TB : TB + TK, :]
                )
                for j in range(GS):
                    s = g * GS + j
                    if s + 3 < SPC:
                        xtn = xp.tile([128, 3, 2 * XR], f8, name="xtn")
                        xts[s + 3] = xtn
                        nc.sync.dma_start(
                            out=xtn[:], in_=win3(x_d[s + 3], 128, 2 * XR, 121)
                        )
                    if tt is None:
                        tt = tbp.tile([GS * TK, SPC // GS, 2 * KW, 128], f8)
                        nc.sync.dma_start(out=tt[:], in_=t_d[:].transpose([1, 0, 2, 3]))
                    xt = xts.pop(s)
                    btj = bta[:, 0] if s == 0 else bt[:, j]
                    ot = op.tile([121, 3, WO], bf16)
                    for bi, (obase, M, K) in enumerate(MAIN_BLOCKS):
                        ps0 = pa.tile([128, 512], f32)
                        ps2 = pb.tile([128, 512], f32)
                        ps1 = pc.tile([128, 512], f32)
                        for sc, psx in ((0, ps0), (1, ps2)):
                            offs = [2 * (3 - u) + sc for u in range(4)]
                            for u in range(4):
                                xhi = xt[
                                    :K, bi,
                                    offs[u] : offs[u] + 2 * NP - 1 : 2,
                                ]
                                nc.tensor.matmul(
                                    psx[:, :NP],
                                    pair(btj[:K, 4 * sc + u, :], 0),
                                    pair(xhi, XR),
                                    start=(u == 0),
                                    stop=False,
                                    perf_mode=DR,
                                )
                            for pi, (u0, u1) in enumerate(((0, 2), (1, 3))):
                                xhi = xt[
                                    :K, bi,
                                    offs[u0] : offs[u0] + 2 * NP - 1 : 2,
                                ]
                                nc.tensor.matmul(
                                    psx[:, :NP],
                                    pair(btj[:K, 8 + 4 * sc + u0, :], 256),
                                    pair(xhi, offs[u1] - offs[u0]),
                                    start=False,
                                    stop=(pi == 1),
                                    perf_mode=DR,
                                )
                        # P1 = (H0+H1)*(X0+X1): sum H0+H1 on the fly via
                        # weight pairs (plane u, plane 4+u), ifmap stride 0
                        for u in range(4):
                            po = W + 3 - u
                            for vi, voff in enumerate((0, XR)):
                                nc.tensor.matmul(
                                    ps1[:, :NP],
                                    pair(btj[:K, u, :], 512),
                                    pair(xt[:K, bi, voff + po : voff + po + NP], 0),
                                    start=(u == 0 and vi == 0),
                                    stop=False,
                                    perf_mode=DR,
                                )
                        for u in range(4):
                            po = W + 3 - u
                            nc.tensor.matmul(
                                ps1[:, :NP],
                                pair(btj[:K, 8 + u, :], 512),
                                pair(xt[:K, bi, po : po + NP], 0),
                                start=False,
                                stop=(u == 3),
                                perf_mode=DR,
                            )
                        # stage P0/P2 in SBUF (Act), recombine on DVE:
                        # even = P1-P0-P2, odd[t] = P0[t+1]+P2[t]
                        c0 = tp.tile([128, NP], f32)
                        nc.scalar.copy(out=c0[:M, :], in_=ps0[:M, :NP])
                        c2 = tp.tile([128, NP], f32)
                        nc.scalar.copy(out=c2[:M, :], in_=ps2[:M, :NP])
                        nc.vector.tensor_add(
                            out=ot[:M, bi, 1 : WO : 2],
                            in0=c0[:M, 1:NP],
                            in1=c2[:M, 0 : NP - 1],
                        )
                        t0 = tp.tile([128, NP], f32)
                        nc.vector.tensor_sub(
                            out=t0[:M, :], in0=ps1[:M, :NP], in1=c0[:M, :]
                        )
                        nc.vector.tensor_sub(
                            out=ot[:M, bi, 0 : WO : 2], in0=t0[:M, :], in1=c2[:M, :]
                        )
                    if s == SPC - 1:
                        for bi2, (ob2, M2, _K2) in enumerate(MAIN_BLOCKS):
                            nc.sync.dma_start(
                                out=o_d[s, ob2 : ob2 + M2, :],
                                in_=ot[:M2, bi2, :],
                            )
                    else:
                        nc.gpsimd.dma_start(
                            out=win3(o_d[s], 121, WO, 121), in_=ot[:]
                        )
                    if j == 0 and g + 1 < SPC // GS:
                        g2 = g + 1
                        btn = bp.tile([128, GS, 16, 128], f8, name="bt")
                        bts[g2] = btn
                        nc.sync.dma_start(
                            out=btn[:],
                            in_=b_d[g2 * GS : (g2 + 1) * GS].transpose([1, 0, 2, 3]),
                        )
                    if j == 1:
                        # tail: GS samples' last 14 rows, block-diagonal matmul
                        ps = pt.tile([128, 512], f32)
                        for q in range(KW):
                            nc.tensor.matmul(
                                ps[:, :WO],
                                pair(tt[: GS * TK, g, q, :], 0),
                                pair(xtt[: GS * TK, q : q + WO], XR),
                                start=(q == 0),
                                stop=False,
                                perf_mode=DR,
                            )
                        for pi, (q0, q1) in enumerate(((0, 2), (1, 3), (4, 6), (5, 7))):
                            nc.tensor.matmul(
                                ps[:, :WO],
                                pair(tt[: GS * TK, g, KW + q0, :], 256),
                                pair(xtt[: GS * TK, q0 : q0 + WO], q1 - q0),
                                start=False,
                                stop=(pi == 3),
                                perf_mode=DR,
                            )
                        oto = op.tile([121, 3, WO], bf16)
                        nc.scalar.copy(
                            out=oto[: GS * TM, 0, :], in_=ps[: GS * TM, :WO]
                        )
                        nc.gpsimd.dma_start(
                            out=o_d[g * GS : (g + 1) * GS, TB : TB + TM, :],
                            in_=oto[: GS * TM, 0, :],
                        )

    nc.compile()
    return nc


def _build_runner():
    """Build nc + a persistent jitted PJRT callable (compiles once)."""
    import jax
    from jax.sharding import Mesh, PartitionSpec
    from jax.experimental.shard_map import shard_map
    import concourse.mybir as mybir
    from concourse import bass2jax

    nc = _build_program()
    bass2jax.install_neuronx_cc_hook()

    partition_name = nc.partition_id_tensor.name if nc.partition_id_tensor else None

    in_names, out_names, out_avals, zero_shapes = [], [], [], []
    for alloc in nc.m.functions[0].allocations:
        if not isinstance(alloc, mybir.MemoryLocationSet):
            continue
        name = alloc.memorylocations[0].name
        if alloc.kind == "ExternalInput":
            if name != partition_name:
                in_names.append(name)
        elif alloc.kind == "ExternalOutput":
            shape = tuple(alloc.tensor_shape)
            dtype = mybir.dt.np(alloc.dtype)
            out_names.append(name)
            out_avals.append(jax.core.ShapedArray(shape, dtype))
            zero_shapes.append((shape, dtype))
    n_params = len(in_names)
    n_outs = len(out_avals)
    all_in_names = list(in_names) + list(out_names)
    if partition_name is not None:
        all_in_names.append(partition_name)

    def _body(*args):
        operands = list(args)
        if partition_name is not None:
            operands.append(bass2jax.partition_id_tensor())
        outs = bass2jax._bass_exec_p.bind(
            *operands,
            out_avals=tuple(out_avals),
            in_names=tuple(all_in_names),
            out_names=tuple(out_names),
            lowering_input_output_aliases=(),
            sim_require_finite=True,
            sim_require_nnan=True,
            nc=nc,
        )
        return tuple(outs)

    devices = jax.devices()[:N_CORES]
    mesh = Mesh(np.asarray(devices), ("core",))
    in_specs = (PartitionSpec("core"),) * (n_params + n_outs)
    out_specs = (PartitionSpec("core"),) * n_outs
    sharded = jax.jit(
        shard_map(
            _body, mesh=mesh, in_specs=in_specs, out_specs=out_specs, check_rep=False
        ),
        keep_unused=True,
    )

    from jax.sharding import NamedSharding

    zero_sharding = NamedSharding(mesh, PartitionSpec("core"))
    dev_zeros = [
        jax.device_put(np.zeros((N_CORES * s[0], *s[1:]), d), zero_sharding)
        for (s, d) in zero_shapes
    ]

    def run(in_maps):
        concat_in = [
            np.concatenate([np.asarray(m[name]) for m in in_maps], axis=0)
            for name in in_names
        ]
        out_arrs = sharded(*concat_in, *dev_zeros)
        return [
            {
                name: np.asarray(out_arrs[i]).reshape(
                    N_CORES, *out_avals[i].shape
                )[c]
                for i, name in enumerate(out_names)
            }
            for c in range(N_CORES)
        ]

    return nc, run


def _build_bands(k2):
    """k2: [B, 8, 8] fp32 -> fp8 hi/lo Karatsuba bands [B, 128, 24, 121].

    fp32 planes 0-3: Toeplitz bands of H0 (even taps of q-reversed k),
    4-7: H1, 8-11: H0+H1; fp8 planes 0-11 = hi split, 12-23 = lo
    residual. bands[b, m+p, plane(u), m] = Hx[b, p, u].
    """
    h = k2[:, :, ::-1]
    H = np.concatenate([h[:, :, 0::2], h[:, :, 1::2]], axis=2)  # [B, 8, 8]
    bands = np.zeros((k2.shape[0], 128, 8, 128), np.float32)
    m = np.arange(121)
    for p in range(KH):
        for u in range(8):
            bands[:, m + p, u, m] = H[:, p, u][:, None]
    hi, lo = _hi_lo(bands)
    return np.concatenate([hi, lo], axis=2)


def _build_tailbands(k2):
    """k2: [N, 8, 8] -> fp8 hi/lo block-diag tail bands [N//GS, GS*21, 16, GS*14]."""
    n = k2.shape[0]
    tb = np.zeros((n // GS, GS * TK, KW, 128), np.float32)
    m = np.arange(TM)
    for g in range(n // GS):
        for j in range(GS):
            for p in range(KH):
                tb[g, TK * j + m + p, :, TM * j + m] = k2[g * GS + j, p, :]
    hi, lo = _hi_lo(tb)
    return np.concatenate([hi, lo], axis=2)


def kernel(x, k):
    x = np.asarray(x, dtype=np.float32).reshape(B, H, W)
    k = np.asarray(k, dtype=np.float32).reshape(B, KH, KW)

    if "runner" not in _cache:
        _cache["runner"] = _build_runner()
    _nc, run = _cache["runner"]

    xs = x[:, :, 0::2] + x[:, :, 1::2]
    row = np.concatenate([x, xs], axis=2)
    xh, xl = _hi_lo(row)
    xb = np.concatenate([xh, xl], axis=2)
    bands = _build_bands(k)
    tailbands = _build_tailbands(k)
    n_groups = SPC // GS
    in_maps = [
        {
            "x": np.ascontiguousarray(xb[c * SPC : (c + 1) * SPC]),
            "bands": bands[c * SPC : (c + 1) * SPC],
            "tailbands": tailbands[c * n_groups : (c + 1) * n_groups],
        }
        for c in range(N_CORES)
    ]
    results = run(in_maps)
    out = np.concatenate([r["out"] for r in results], axis=0)
    return out.astype(np.float32).reshape(B, HO, WO, 1)
